# revision 12
# baseline (speedup 1.0000x reference)
"""Trainium2 Bass kernel for nn_DownBlock_res_dct1 (maxpool 2x2 + truncated
block-DCT low-pass + SE attention + 1x1 conv + two 3x3 convs), data-parallel
over the batch across 8 NeuronCores.

Self-contained: hardcodes all shapes/constants; builds one SPMD Bass module
(one batch item per core), runs via run_bass_kernel_spmd, gathers the full
(8, 128, 192, 192) output.

Per-core layout: partitions p = half*64 + ch, where half selects image row
halves. Phase A (input DMA / maxpool on GpSimd / DCT+gather+stats on DVE+Act)
is pipelined per row-chunk behind the input DMA stream; phase B (SE gamma ->
att 1x1 + conv1 as concurrent K=64 row-tile matmul pairs, conv2 K=128) keeps
the PE streaming back-to-back. Matmuls in bf16 with fp32 PSUM accumulation.
"""

import math
from collections import deque
from contextlib import ExitStack

import numpy as np

import concourse.bass as bass
import concourse.mybir as mybir
import concourse.tile as tile
from concourse import bacc
from concourse.bass_utils import run_bass_kernel_spmd

FP32 = mybir.dt.float32
BF16 = mybir.dt.bfloat16
AX = mybir.AxisListType
OP = mybir.AluOpType
ACT = mybir.ActivationFunctionType

N = 8  # DCT block size
_P8 = np.arange(8)
COS1 = np.cos(math.pi * (_P8 + 0.5) / 8.0 * 1).astype(np.float64)
COS2 = np.cos(math.pi * (_P8 + 0.5) / 8.0 * 2).astype(np.float64)
# Selected zigzag coeffs [0,1,2,5] -> (k1,k2) in {(0,0),(0,1),(1,0),(0,2)}
A00 = (1.0 / 8.0) ** 2
A01 = 2.0 / 64.0
A02 = 2.0 / 64.0
A10 = 2.0 / 64.0


def _runs(idx):
    """Contiguous runs where idx[i] = i - g: list of (out_start, in_start, len)."""
    runs = []
    s = 0
    for i in range(1, len(idx) + 1):
        if i == len(idx) or idx[i] != idx[i - 1] + 1:
            runs.append((s, int(idx[s]), i - s))
            s = i
    return runs


def _split_at(ro, rin, rl, bound):
    """Split a run at source-row `bound`."""
    if rin < bound < rin + rl:
        return [(ro, rin, bound - rin), (ro + bound - rin, bound, rin + rl - bound)]
    return [(ro, rin, rl)]


def build_nc(H=384, W=384, debug=False):
    C, C2 = 64, 128
    h, w = H // 2, W // 2
    hh = h // 2  # rows per half
    assert hh % N == 0 and w % N == 0
    T = hh // N  # block-rows per half
    S = w // N  # block-cols
    CH = 64

    hi = (np.arange(h) * (h - (N - 1))) // h
    wi = (np.arange(w) * (w - (N - 1))) // w
    col_runs = _runs(wi)
    row_runs_h = [_runs(hi[hh * hf : hh * (hf + 1)]) for hf in (0, 1)]

    MP_CH = 4  # pooled rows per maxpool chunk
    n_mp = hh // MP_CH
    DCT_T = 2  # block-rows per DCT chunk
    n_dct = T // DCT_T
    XPC = DCT_T * N  # xp rows per DCT chunk
    mp_per_xpc = XPC // MP_CH
    assert XPC % MP_CH == 0

    # First half-1 row whose gather source is still in half 0 (cross-half
    # bounce, available only once the last DCT chunk is done).
    j0 = int(np.argmax(hi[hh : 2 * hh] >= hh))
    assert 0 < j0 < XPC
    # (row_start, row_len) local y1 row ranges completed per DCT chunk
    chunk_ranges = {c: [] for c in range(n_dct)}
    chunk_ranges[0].append((j0, XPC - j0))
    for c in range(1, n_dct):
        chunk_ranges[c].append((c * XPC, XPC))
    chunk_ranges[n_dct - 1].append((0, j0))
    n_ranges = sum(len(v) for v in chunk_ranges.values())

    # Gather copy plan: (batch_chunk, hf, dst_row, src_hf, src_local, len)
    gather_plan = {c: [] for c in range(n_dct)}
    for hf in (0, 1):
        for ro, rin_g, rl in row_runs_h[hf]:
            for ro2, rin2, rl2 in _split_at(ro, rin_g, rl, hh):
                src_hf = 0 if rin2 < hh else 1
                cross = src_hf != hf
                r = 0
                while r < rl2:
                    dst0 = ro2 + r
                    take = min(XPC - (dst0 % XPC), rl2 - r)
                    src0 = rin2 + r - hh * src_hf
                    if cross:
                        batch = n_dct - 1
                    else:
                        batch = max(dst0 // XPC, (src0 + take - 1) // XPC)
                    gather_plan[batch].append((hf, dst0, src_hf, src0, take))
                    r += take

    nc = bacc.Bacc("TRN2")

    x = nc.dram_tensor("x", [C, H, W], FP32, kind="ExternalInput")
    w1 = nc.dram_tensor("w1", [C2, C, 3, 3], FP32, kind="ExternalInput")
    b1 = nc.dram_tensor("b1", [C2], FP32, kind="ExternalInput")
    w2 = nc.dram_tensor("w2", [C2, C2, 3, 3], FP32, kind="ExternalInput")
    b2 = nc.dram_tensor("b2", [C2], FP32, kind="ExternalInput")
    attw = nc.dram_tensor("att_conv_w", [C, C, 1, 1], FP32, kind="ExternalInput")
    attb = nc.dram_tensor("att_conv_b", [C], FP32, kind="ExternalInput")
    fc1 = nc.dram_tensor("fc1_w", [C // 16, C, 1, 1], FP32, kind="ExternalInput")
    fc2 = nc.dram_tensor("fc2_w", [C, C // 16, 1, 1], FP32, kind="ExternalInput")
    out = nc.dram_tensor("out", [C2, h, w], FP32, kind="ExternalOutput")

    dbg = {}
    if debug:
        for name, shape in [
            ("dbg_xp", [128, hh, w]),
            ("dbg_y1", [128, hh, w]),
            ("dbg_se", [64, 8]),
            ("dbg_gamma", [64, 1]),
            ("dbg_xall", [128, hh + 2, w + 2]),
            ("dbg_o1", [C2, h + 2, w + 2]),
        ]:
            dbg[name] = nc.dram_tensor(name, shape, FP32, kind="ExternalOutput")

    const_np = np.zeros((128, 4, 8), np.float32)
    const_np[:, 0, :] = COS1
    const_np[:, 1, :] = COS2
    const_np[:, 2, :] = COS1 * A01
    const_np[:, 3, :] = COS2 * A02
    cdram = nc.inline_tensor(const_np.reshape(128, 32), name="dctconst")

    NF = h * w  # pixels per full channel image

    with tile.TileContext(nc) as tc, ExitStack() as ctx:
        wpool = ctx.enter_context(tc.tile_pool(name="wpool", bufs=1))
        if debug:
            dpool = ctx.enter_context(tc.tile_pool(name="dpool", bufs=1))
        smallD = ctx.enter_context(tc.tile_pool(name="smallD", bufs=1))
        small = ctx.enter_context(tc.tile_pool(name="small", bufs=2))
        psA = ctx.enter_context(tc.tile_pool(name="psA", bufs=2, space="PSUM"))
        psC = ctx.enter_context(tc.tile_pool(name="psC", bufs=3, space="PSUM"))
        # phase-scoped pools; each SBUF side is a LIFO stack.
        py1 = tc.alloc_tile_pool(name="py1", bufs=1, side="right")
        pxp = tc.alloc_tile_pool(name="pxp", bufs=1, side="right")
        pin = tc.alloc_tile_pool(name="pin", bufs=2, side="right")
        pws = tc.alloc_tile_pool(name="pws", bufs=1)  # weight staging, short-lived

        # ---------------- constants / weights ----------------
        consts = wpool.tile([128, 4, 8], FP32)
        nc.sync.dma_start(consts[:], cdram[:].rearrange("p (a b) -> p a b", a=4))

        def cvec(row, shp):  # broadcast [128,8] const row to shp (q innermost)
            return consts[:, row, None, None, :].to_broadcast(shp)

        from concourse.masks import make_identity

        ident = wpool.tile([128, 128], FP32)
        make_identity(nc, ident[:])

        zerot = wpool.tile([128, 1], FP32)
        nc.vector.memset(zerot[:], 0.0)

        w1s = pws.tile([C2, C * 9], FP32)
        nc.sync.dma_start(w1s[:], w1[:].rearrange("o i ky kx -> o (i ky kx)"))
        w1t = wpool.tile([128, 9, C2], BF16)
        for tap in range(9):
            pt = psA.tile([C, C2], FP32, tag="ps")
            sv = w1s[:].rearrange("o (i t) -> o t i", t=9)[:, tap, :]
            nc.tensor.transpose(pt[:], sv, ident[:])
            nc.vector.tensor_copy(w1t[0:CH, tap, :], pt[:])
            nc.vector.tensor_copy(w1t[CH:128, tap, :], pt[:])

        w2s = pws.tile([C2, C2 * 9], FP32)
        nc.sync.dma_start(w2s[:], w2[:].rearrange("o i ky kx -> o (i ky kx)"))
        w2t = wpool.tile([128, 9, C2], BF16)
        for tap in range(9):
            pt = psA.tile([C2, C2], FP32, tag="ps")
            sv = w2s[:].rearrange("o (i t) -> o t i", t=9)[:, tap, :]
            nc.tensor.transpose(pt[:], sv, ident[:])
            nc.vector.tensor_copy(w2t[:, tap, :], pt[:])

        atts = pws.tile([C, C], FP32)
        nc.sync.dma_start(atts[:], attw[:, :, 0, 0])
        attt = wpool.tile([128, C], BF16)
        pt = psA.tile([C, C], FP32, tag="ps")
        nc.tensor.transpose(pt[:], atts[:], ident[0:C, 0:C])
        nc.vector.tensor_copy(attt[0:CH, :], pt[:])
        nc.vector.tensor_copy(attt[CH:128, :], pt[:])

        fc1t = pws.tile([C, C // 16], FP32)
        nc.sync.dma_start(fc1t[:], fc1[:, :, 0, 0].rearrange("o c -> c o"))
        fc1b = wpool.tile([C, C // 16], BF16)
        nc.vector.tensor_copy(fc1b[:], fc1t[:])
        fc2t = pws.tile([C // 16, C], FP32)
        nc.sync.dma_start(fc2t[:], fc2[:, :, 0, 0].rearrange("o c -> c o"))
        fc2b = wpool.tile([C // 16, C], BF16)
        nc.vector.tensor_copy(fc2b[:], fc2t[:])

        b1t = wpool.tile([C2, 1], FP32)
        nc.sync.dma_start(b1t[:], b1[:, None])
        b2t = wpool.tile([C2, 1], FP32)
        nc.sync.dma_start(b2t[:], b2[:, None])
        attbt = wpool.tile([C, 1], FP32)
        nc.sync.dma_start(attbt[:], attb[:, None])
        attg = wpool.tile([128, C], BF16)  # gamma-folded att weights (phase B)

        pws.release()

        pxa = tc.alloc_tile_pool(name="pxa", bufs=1)
        prec = tc.alloc_tile_pool(name="prec", bufs=1)

        # ---------------- phase A tiles ----------------
        x_all = pxa.tile([128, hh + 2, w + 2], BF16)
        nc.gpsimd.memset(x_all[:, :, 0], 0.0)
        nc.gpsimd.memset(x_all[:, :, w + 1], 0.0)
        nc.gpsimd.memset(x_all[0:CH, 0, :], 0.0)
        nc.gpsimd.memset(x_all[CH:128, hh + 1, :], 0.0)

        recon = prec.tile([128, hh, w], BF16)  # pre-gather reconstruction
        y1 = py1.tile([128, hh, w], BF16)
        xp_tiles = [
            pxp.tile([128, XPC, w], BF16, tag=f"xp{i}", name=f"xp{i}")
            for i in range(n_dct)
        ]
        ysum_t = small.tile([128, n_ranges], FP32, tag="ysum", bufs=1)
        ysq_t = small.tile([128, n_ranges], FP32, tag="ysq", bufs=1)

        lowp = nc.allow_low_precision(reason="bf16 DCT partials, ample tolerance")
        lowp.__enter__()

        # ---------------- emission helpers (interleaved pipeline) ----------
        shp4 = (128, DCT_T, S, N)

        def dct_ops(c):
            """DCT of xp chunk c -> recon chunk c; list of closures."""
            xpt = xp_tiles[c]
            ops = []

            def f_a0():
                # tree-sum the 8 rows per block (packed bf16 -> DVE 2x mode)
                xv = xpt[:].rearrange("p (t r) q -> p t r q", r=N)
                t1 = smallD.tile([128, DCT_T, 4, w], BF16, tag="t1", name="t1")
                nc.vector.tensor_tensor(
                    t1[:], xv[:, :, 0:4, :], xv[:, :, 4:8, :], OP.add
                )
                t2 = smallD.tile([128, DCT_T, 2, w], BF16, tag="t2", name="t2")
                nc.vector.tensor_tensor(
                    t2[:], t1[:, :, 0:2, :], t1[:, :, 2:4, :], OP.add
                )
                a0 = smallD.tile([128, DCT_T, w], BF16, tag="a0", name="a0")
                nc.vector.tensor_tensor(
                    a0[:], t2[:, :, 0, :], t2[:, :, 1, :], OP.add
                )
                return a0

            def f_qs():
                qs = smallD.tile([128, DCT_T * N, S], BF16, tag="qs", name="qs")
                nc.vector.tensor_reduce(
                    qs[:], xpt[:].rearrange("p tr (s q) -> p tr s q", q=N),
                    axis=AX.X, op=OP.add,
                )
                return qs

            st = {}
            ops.append(lambda: st.__setitem__("a0", f_a0()))
            ops.append(lambda: st.__setitem__("qs", f_qs()))

            def f_c10():
                c10 = smallD.tile([128, DCT_T, S], FP32, tag="c10", name="c10")
                qsv = st["qs"][:].rearrange("p (t r) s -> p t r s", r=N)
                nc.vector.tensor_scalar(
                    c10[:], qsv[:, :, 0, :], float(COS1[0]), None, OP.mult
                )
                for r in range(1, N):
                    nc.vector.scalar_tensor_tensor(
                        c10[:], qsv[:, :, r, :], float(COS1[r]), c10[:],
                        OP.mult, OP.add,
                    )
                st["c10"] = c10

            ops.append(f_c10)

            def f_cX():
                a0 = st["a0"]
                a0v = a0[:].rearrange("p t (s q) -> p t s q", q=N)
                c00 = smallD.tile([128, DCT_T, S], BF16, tag="c00", name="c00")
                nc.vector.tensor_reduce(c00[:], a0v, axis=AX.X, op=OP.add)
                tmp = smallD.tile([128, DCT_T, w], BF16, tag="tmp", name="tmp")
                tmpv = tmp[:].rearrange("p t (s q) -> p t s q", q=N)
                nc.vector.tensor_tensor(tmpv, a0v, cvec(2, shp4), OP.mult)
                c01 = smallD.tile([128, DCT_T, S], BF16, tag="c01", name="c01")
                nc.vector.tensor_reduce(c01[:], tmpv, axis=AX.X, op=OP.add)
                nc.vector.tensor_tensor(tmpv, a0v, cvec(3, shp4), OP.mult)
                c02 = smallD.tile([128, DCT_T, S], BF16, tag="c02", name="c02")
                nc.vector.tensor_reduce(c02[:], tmpv, axis=AX.X, op=OP.add)
                st["c00"], st["c01"], st["c02"] = c00, c01, c02

            ops.append(f_cX)

            def f_e0():
                e0 = smallD.tile([128, DCT_T, w], BF16, tag="e0", name="e0")
                e0v = e0[:].rearrange("p t (s q) -> p t s q", q=N)
                tmp8 = smallD.tile([128, DCT_T, w], BF16, tag="tmp", name="tmp8")
                tmp8v = tmp8[:].rearrange("p t (s q) -> p t s q", q=N)
                c01b = st["c01"][:, :, :, None].to_broadcast(shp4)
                c02b = st["c02"][:, :, :, None].to_broadcast(shp4)
                c00b = st["c00"][:, :, :, None].to_broadcast(shp4)
                nc.vector.tensor_tensor(e0v, c01b, cvec(0, shp4), OP.mult)
                nc.vector.tensor_tensor(tmp8v, c02b, cvec(1, shp4), OP.mult)
                nc.vector.tensor_tensor(e0[:], e0[:], tmp8[:], OP.add)
                nc.vector.scalar_tensor_tensor(
                    e0v, c00b, A00, e0v, OP.mult, OP.add
                )
                st["e0"] = e0
                # broadcast c10 across q on the scalar engine (keeps DVE 2x)
                c10e = smallD.tile([128, DCT_T, w], BF16, tag="c10e", name="c10e")
                c10ev = c10e[:].rearrange("p t (s q) -> p t s q", q=N)
                nc.scalar.copy(c10ev, st["c10"][:, :, :, None].to_broadcast(shp4))
                st["c10e"] = c10e

            ops.append(f_e0)

            rv = recon[:, c * XPC : (c + 1) * XPC, :].rearrange(
                "p (t r) q -> p t r q", r=N
            )

            def mk_recon(r):
                def f():
                    nc.vector.scalar_tensor_tensor(
                        rv[:, :, r, :], st["c10e"][:], float(A10 * COS1[r]),
                        st["e0"][:], OP.mult, OP.add,
                    )
                return f

            for r in range(N):
                ops.append(mk_recon(r))
            return ops

        def gather_ops(c):
            """Gather batch c: recon -> y1 row copies; DVE for half0 dst,
            scalar engine for half1 dst."""
            ops = []

            def mk(hf, dst0, src_hf, src0, take):
                def f():
                    pb = hf * CH
                    pbi = src_hf * CH
                    if pbi != pb:
                        # cross-half rows: bounce through DMA into a
                        # base-aligned staging tile
                        assert take <= j0
                        xstage = small.tile(
                            [128, j0, w], BF16, tag="xstage", name="xstage"
                        )
                        nc.sync.dma_start(
                            xstage[pb : pb + CH, 0:take, :],
                            recon[pbi : pbi + CH, src0 : src0 + take, :],
                        )
                        srct, srow, spb = xstage, 0, pb
                    else:
                        srct, srow, spb = recon, src0, pbi
                    for co, cin, cl in col_runs:
                        src = srct[spb : spb + CH, srow : srow + take, cin : cin + cl]
                        dst = y1[pb : pb + CH, dst0 : dst0 + take, co : co + cl]
                        if hf == 1:
                            nc.scalar.copy(dst, src)
                        else:
                            nc.gpsimd.tensor_copy(dst, src)
                return f

            for args in gather_plan[c]:
                ops.append(mk(*args))
            return ops

        def tail_ops(c):
            """y2 into x_all + SE stat partials for ranges completed at c."""
            ops = []

            def mk(slot, r0, rl):
                def f():
                    ti, lr = r0 // XPC, r0 % XPC
                    xpv = xp_tiles[ti][:, lr : lr + rl, :]
                    nc.gpsimd.tensor_tensor(
                        x_all[:, 1 + r0 : 1 + r0 + rl, 1 : w + 1],
                        xpv, y1[:, r0 : r0 + rl, :], OP.subtract,
                    )
                    # stat partials; dumps go to dead storage (recon / xp)
                    nc.scalar.activation(
                        recon[:, r0 : r0 + rl, :], y1[:, r0 : r0 + rl, :],
                        ACT.Copy, accum_out=ysum_t[:, slot : slot + 1],
                    )
                    nc.scalar.activation(
                        xpv, y1[:, r0 : r0 + rl, :],
                        ACT.Square, accum_out=ysq_t[:, slot : slot + 1],
                    )
                return f

            for r0, rl in chunk_ranges[c]:
                ops.append(mk(tail_ops.slot, r0, rl))
                tail_ops.slot += 1
            return ops

        tail_ops.slot = 0

        # ---------------- load + maxpool + pipelined phase A ----------------
        pending = deque()

        def drain(k):
            for _ in range(k):
                if not pending:
                    return
                pending.popleft()()

        OPS_PER_MP = 4
        for k in range(n_mp):
            xin = pin.tile([128, 2 * MP_CH, W], FP32, tag="xin", name="xin")
            r0 = 2 * MP_CH * k
            nc.sync.dma_start(xin[0:CH, :, :], x[:, r0 : r0 + 2 * MP_CH, :])
            nc.sync.dma_start(
                xin[CH:128, :, :], x[:, H // 2 + r0 : H // 2 + r0 + 2 * MP_CH, :]
            )
            hmax = pin.tile([128, 2 * MP_CH, w], BF16, tag="hmax", name="hmax",
                            bufs=1)
            xv = xin[:].rearrange("p r (a two) -> p r a two", two=2)
            nc.vector.tensor_tensor(hmax[:], xv[:, :, :, 0], xv[:, :, :, 1], OP.max)
            xpt = xp_tiles[k // mp_per_xpc]
            rr = (k % mp_per_xpc) * MP_CH
            hv = hmax[:].rearrange("p (b two) q -> p b two q", two=2)
            nc.vector.tensor_tensor(
                xpt[:, rr : rr + MP_CH, :], hv[:, :, 0, :], hv[:, :, 1, :], OP.max
            )
            drain(OPS_PER_MP)
            if (k + 1) % mp_per_xpc == 0:
                c = k // mp_per_xpc
                pending.extend(dct_ops(c))
                pending.extend(gather_ops(c))
                # stats/y2 of chunk c-1: deferred one batch so the recon dump
                # never clobbers rows batch c's gather still reads
                if c > 0:
                    pending.extend(tail_ops(c - 1))
        pending.extend(tail_ops(n_dct - 1))
        drain(len(pending) + 1)

        pin.release()

        if debug:
            xpd = dpool.tile([128, hh, w], FP32, tag="xpd")
            for c in range(n_dct):
                nc.vector.tensor_copy(
                    xpd[:, c * XPC : (c + 1) * XPC, :], xp_tiles[c][:]
                )
            nc.sync.dma_start(dbg["dbg_xp"][:], xpd[:])
            y1d = dpool.tile([128, hh, w], FP32, tag="y1d")
            nc.vector.tensor_copy(y1d[:], y1[:])
            nc.sync.dma_start(dbg["dbg_y1"][:], y1d[:])

        # ---------------- SE ----------------
        ysum = small.tile([128, 1], FP32, tag="ysumT", bufs=1)
        ysq = small.tile([128, 1], FP32, tag="ysqT", bufs=1)
        nc.vector.tensor_reduce(
            ysum[:], ysum_t[:, None, :], axis=AX.X, op=OP.add
        )
        nc.vector.tensor_reduce(ysq[:], ysq_t[:, None, :], axis=AX.X, op=OP.add)

        st = small.tile([64, 12], FP32, tag="se", bufs=1)
        yhi = small.tile([64, 2], FP32, tag="yhi", bufs=1)
        nc.sync.dma_start(yhi[:, 0:1], ysum[CH:128, :])
        nc.sync.dma_start(yhi[:, 1:2], ysq[CH:128, :])
        nc.vector.tensor_tensor(st[:, 0:1], ysum[0:CH, :], yhi[:, 0:1], OP.add)
        nc.vector.tensor_tensor(st[:, 1:2], ysq[0:CH, :], yhi[:, 1:2], OP.add)
        nc.vector.tensor_scalar(st[:, 2:3], st[:, 0:1], 1.0 / NF, None, OP.mult)
        nc.vector.tensor_scalar(st[:, 3:4], st[:, 1:2], 1.0 / NF, None, OP.mult)
        nc.vector.tensor_tensor(st[:, 4:5], st[:, 2:3], st[:, 2:3], OP.mult)
        nc.vector.tensor_tensor(st[:, 5:6], st[:, 3:4], st[:, 4:5], OP.subtract)
        nc.vector.tensor_scalar(
            st[:, 6:7], st[:, 5:6], float(NF) / float(NF - 1), None, OP.mult
        )
        nc.vector.tensor_tensor(st[:, 7:8], st[:, 2:3], st[:, 6:7], OP.add)
        sb = small.tile([64, 1], BF16, tag="sb16", bufs=1)
        nc.vector.tensor_copy(sb[:], st[:, 7:8])
        pfc1 = psA.tile([C // 16, 1], FP32, tag="ps")
        nc.tensor.matmul(pfc1[:], fc1b[:], sb[:], start=True, stop=True)
        tb = small.tile([C // 16, 1], BF16, tag="tb16", bufs=1)
        nc.scalar.activation(tb[:], pfc1[:], ACT.Relu)
        pfc2 = psA.tile([C, 1], FP32, tag="ps")
        nc.tensor.matmul(pfc2[:], fc2b[:], tb[:], start=True, stop=True)
        gamma = small.tile([64, 1], FP32, tag="gamma", bufs=1)
        nc.scalar.activation(gamma[:], pfc2[:], ACT.Sigmoid)
        gamma128 = small.tile([128, 1], FP32, tag="g128", bufs=1)
        nc.vector.tensor_copy(gamma128[0:CH, :], gamma[:])
        nc.sync.dma_start(gamma128[CH:128, :], gamma[:])
        # fold gamma into the att weights: rhs of the att conv is then y1
        nc.vector.tensor_scalar(attg[:], attt[:], gamma128[:, 0:1], None, OP.mult)
        if debug:
            nc.sync.dma_start(dbg["dbg_se"][:], st[:, 0:8])
            nc.sync.dma_start(dbg["dbg_gamma"][:], gamma[:])

        # ---------------- att conv (concurrent K=64 half pairs) ------------
        ATT_G = 8
        n_att_g = hh // ATT_G
        FLAT = ATT_G * w
        AN = 512
        n_fl = FLAT // AN
        y1v0 = y1[0:CH, :, :].rearrange("p a b -> p (a b)")
        y1v1 = y1[CH:128, :, :].rearrange("p a b -> p (a b)")
        for g in range(n_att_g):
            xc = small.tile([128, ATT_G, w], BF16, tag="xc", name="xc")
            xcv0 = xc[0:CH, :, :].rearrange("p a b -> p (a b)")
            xcv1 = xc[CH:128, :, :].rearrange("p a b -> p (a b)")
            base = g * FLAT
            for f in range(n_fl):
                paA = psC.tile([CH, AN], FP32, tag="ps0", name="paA")
                paB = psC.tile([CH, AN], FP32, tag="ps1", name="paB")
                sl = slice(base + f * AN, base + (f + 1) * AN)
                nc.tensor.matmul(
                    paA[:], attg[0:CH, :], y1v0[:, sl], start=True, stop=True
                )
                nc.tensor.matmul(
                    paB[:], attg[CH:128, :], y1v1[:, sl], start=True, stop=True
                )
                fsl = slice(f * AN, (f + 1) * AN)
                nc.scalar.activation(
                    xcv0[:, fsl], paA[:], ACT.Relu, bias=attbt[:, 0:1]
                )
                nc.scalar.activation(
                    xcv1[:, fsl], paB[:], ACT.Relu, bias=attbt[:, 0:1]
                )
            sl2 = x_all[:, 1 + g * ATT_G : 1 + (g + 1) * ATT_G, 1 : w + 1]
            nc.vector.tensor_tensor(sl2, sl2, xc[:], OP.add)

        # cross-half halo rows (needs fully assembled x_all interior)
        nc.sync.dma_start(x_all[CH:128, 0, :], x_all[0:CH, hh, :])
        nc.sync.dma_start(x_all[0:CH, hh + 1, :], x_all[CH:128, 1, :])
        if debug:
            xad = dpool.tile([128, hh + 2, w + 2], FP32, tag="xad")
            nc.vector.tensor_copy(xad[:], x_all[:])
            nc.sync.dma_start(dbg["dbg_xall"][:], xad[:])

        # ---------------- conv1 -> o1 (concurrent K=64 half pairs) ---------
        prec.release()
        pxp.release()
        po1 = tc.alloc_tile_pool(name="po1", bufs=1)
        o1 = po1.tile([C2, h + 2, w + 2], BF16)
        nc.gpsimd.memset(o1[:, 0, :], 0.0)
        nc.gpsimd.memset(o1[:, h + 1, :], 0.0)
        nc.gpsimd.memset(o1[:, :, 0], 0.0)
        nc.gpsimd.memset(o1[:, :, w + 1], 0.0)

        RT = 2
        n_c1 = hh // RT
        # halo-dependent groups (h0 last, h1 first) go in the final pair
        pairs = [(g, g + 1) for g in range(n_c1 - 1)] + [(n_c1 - 1, 0)]
        for ga, gb in pairs:
            pcA = psC.tile([C2, RT * w], FP32, tag="ps0", name="pcA")
            pcB = psC.tile([C2, RT * w], FP32, tag="ps1", name="pcB")
            la, lb = ga * RT, gb * RT
            for tap in range(9):
                dy, dx = divmod(tap, 3)
                nc.tensor.matmul(
                    pcA[:], w1t[0:CH, tap, :],
                    x_all[0:CH, la + dy : la + dy + RT, dx : dx + w],
                    start=(tap == 0), stop=(tap == 8),
                )
                nc.tensor.matmul(
                    pcB[:], w1t[CH:128, tap, :],
                    x_all[CH:128, lb + dy : lb + dy + RT, dx : dx + w],
                    start=(tap == 0), stop=(tap == 8),
                )
            dstA = o1[:, 1 + la : 1 + la + RT, 1 : w + 1]
            nc.scalar.activation(dstA, pcA[:], ACT.Relu, bias=b1t[:, 0:1])
            dstB = o1[:, 1 + hh + lb : 1 + hh + lb + RT, 1 : w + 1]
            nc.vector.scalar_tensor_tensor(
                dstB, pcB[:], b1t[:, 0:1],
                zerot[:, 0:1, None].to_broadcast((C2, RT, w)),
                OP.add, OP.max,
            )
        if debug:
            o1d = dpool.tile([C2, h + 2, w + 2], FP32, tag="o1d")
            nc.vector.tensor_copy(o1d[:], o1[:])
            nc.sync.dma_start(dbg["dbg_o1"][:], o1d[:])

        # ---------------- conv2 -> out ----------------
        n_c2 = h // RT
        # groups touching the last-emitted conv1 pair's o1 rows go last
        defer = [g for g in range(n_c2) if 2 * g + RT + 2 >= 2 * (n_c1 - 1) + 1
                 and 2 * g <= hh + 2]
        order = [g for g in range(n_c2) if g not in defer] + defer
        for i, g in enumerate(order):
            pc = psC.tile([C2, RT * w], FP32, tag=("ps0" if i % 2 == 0 else "ps1"),
                          name="pc2")
            lr = g * RT
            for tap in range(9):
                dy, dx = divmod(tap, 3)
                rhs = o1[:, lr + dy : lr + dy + RT, dx : dx + w]
                nc.tensor.matmul(
                    pc[:], w2t[:, tap, :], rhs, start=(tap == 0), stop=(tap == 8)
                )
            stg = small.tile([C2, RT * w], FP32, tag="ostg", name="ostg")
            if i % 2 == 0:
                nc.scalar.activation(stg[:], pc[:], ACT.Relu, bias=b2t[:, 0:1])
            else:
                nc.vector.scalar_tensor_tensor(
                    stg[:], pc[:], b2t[:, 0:1],
                    zerot[:, 0:1].to_broadcast((C2, RT * w)),
                    OP.add, OP.max,
                )
            nc.sync.dma_start(out[:, lr : lr + RT, :], stg[:])

        lowp.__exit__(None, None, None)
        po1.release()
        pxa.release()
        py1.release()

    nc.finalize()
    return nc


_NC_CACHE = {}


def _get_nc(H=384, W=384, debug=False):
    key = (H, W, debug)
    if key not in _NC_CACHE:
        _NC_CACHE[key] = build_nc(H=H, W=W, debug=debug)
    return _NC_CACHE[key]


def kernel(x, w1, b1, w2, b2, att_conv_w, att_conv_b, fc1_w, fc2_w):
    x = np.ascontiguousarray(np.asarray(x, np.float32))
    B = x.shape[0]
    nc = _get_nc(x.shape[2], x.shape[3])
    shared = {
        "w1": np.ascontiguousarray(np.asarray(w1, np.float32)),
        "b1": np.ascontiguousarray(np.asarray(b1, np.float32)),
        "w2": np.ascontiguousarray(np.asarray(w2, np.float32)),
        "b2": np.ascontiguousarray(np.asarray(b2, np.float32)),
        "att_conv_w": np.ascontiguousarray(np.asarray(att_conv_w, np.float32)),
        "att_conv_b": np.ascontiguousarray(np.asarray(att_conv_b, np.float32)),
        "fc1_w": np.ascontiguousarray(np.asarray(fc1_w, np.float32)),
        "fc2_w": np.ascontiguousarray(np.asarray(fc2_w, np.float32)),
    }
    in_maps = [dict(shared, x=np.ascontiguousarray(x[i])) for i in range(B)]
    res = run_bass_kernel_spmd(nc, in_maps, core_ids=list(range(B)))
    return np.stack([res.results[i]["out"] for i in range(B)], axis=0)


# revision 19
# speedup vs baseline: 1.0293x; 1.0293x over previous
"""Trainium2 Bass kernel for nn_DownBlock_res_dct1 (maxpool 2x2 + truncated
block-DCT low-pass + SE attention + 1x1 conv + two 3x3 convs), data-parallel
over the batch across 8 NeuronCores.

Self-contained: hardcodes all shapes/constants; builds one SPMD Bass module
(one batch item per core), runs via run_bass_kernel_spmd, gathers the full
(8, 128, 192, 192) output.

Layout: partitions p = half*64 + ch (image row halves). Phase A streams the
input DMA while maxpool (into x_all directly) + DCT coefficients + SE stats
run on DVE/GpSimd. The SE statistics (mean/var of y1) are computed in
COEFFICIENT space: sum(y1) and sum(y1^2) are linear/bilinear functionals of
the 4 kept DCT coefficients with per-block weight tables that fold in the
nearest-resize row/col duplication — so gamma is ready right after the last
coefficient chunk, without materializing y1.

Phase B per 16-row chunk: recon from coefficients, att 1x1 conv computed ON
recon (xcr = relu(attg.recon + b), gamma folded into weights), D = recon -
xcr in place, then x_all -= D[gathered] (fused gather-subtract; x_all already
holds the pooled image). conv1 runs as concurrent K=64 row-tile matmul pairs
(partition halves at PE tile positions (0,0)/(64,0)), conv2 as a K=128
stream, scheduled per-chunk behind the gather so the PE never starves.
"""

import math
from collections import deque
from contextlib import ExitStack

import numpy as np

import concourse.bass as bass
import concourse.mybir as mybir
import concourse.tile as tile
from concourse import bacc
from concourse.bass_utils import run_bass_kernel_spmd

FP32 = mybir.dt.float32
BF16 = mybir.dt.bfloat16
AX = mybir.AxisListType
OP = mybir.AluOpType
ACT = mybir.ActivationFunctionType

N = 8  # DCT block size
_P8 = np.arange(8)
COS1 = np.cos(math.pi * (_P8 + 0.5) / 8.0 * 1).astype(np.float64)
COS2 = np.cos(math.pi * (_P8 + 0.5) / 8.0 * 2).astype(np.float64)
# Selected zigzag coeffs [0,1,2,5] -> (k1,k2) in {(0,0),(0,1),(1,0),(0,2)}
A00 = (1.0 / 8.0) ** 2
A01 = 2.0 / 64.0
A02 = 2.0 / 64.0
A10 = 2.0 / 64.0


def _runs(idx):
    """Contiguous runs where idx[i] = i - g: list of (out_start, in_start, len)."""
    runs = []
    s = 0
    for i in range(1, len(idx) + 1):
        if i == len(idx) or idx[i] != idx[i - 1] + 1:
            runs.append((s, int(idx[s]), i - s))
            s = i
    return runs


def _split_at(ro, rin, rl, bound):
    """Split a run at source-row `bound`."""
    if rin < bound < rin + rl:
        return [(ro, rin, bound - rin), (ro + bound - rin, bound, rin + rl - bound)]
    return [(ro, rin, rl)]


def build_nc(H=384, W=384, debug=False):
    C, C2 = 64, 128
    h, w = H // 2, W // 2
    hh = h // 2  # rows per half
    assert hh % N == 0 and w % N == 0
    T = hh // N  # block-rows per half
    S = w // N  # block-cols
    CH = 64

    hi = (np.arange(h) * (h - (N - 1))) // h
    wi = (np.arange(w) * (w - (N - 1))) // w
    col_runs = _runs(wi)
    row_runs_h = [_runs(hi[hh * hf : hh * (hf + 1)]) for hf in (0, 1)]

    MP_CH = 4  # pooled rows per maxpool chunk
    n_mp = hh // MP_CH
    DCT_T = 2  # block-rows per chunk
    n_dct = T // DCT_T
    XPC = DCT_T * N  # pooled rows per chunk
    mp_per_xpc = XPC // MP_CH
    assert XPC % MP_CH == 0
    RT = 2  # conv output rows per matmul group
    n_c1 = hh // RT

    # first half-1 row whose gather source is still in half 0
    j0 = int(np.argmax(hi[hh : 2 * hh] >= hh))
    assert 0 < j0 < XPC

    # Gather plan: (batch, hf, dst_row, src_hf, src_local, len). Batch c means
    # the subtract runs after D-chunk c exists (src rows <= chunk c).
    gather_plan = {c: [] for c in range(n_dct)}
    for hf in (0, 1):
        for ro, rin_g, rl in row_runs_h[hf]:
            for ro2, rin2, rl2 in _split_at(ro, rin_g, rl, hh):
                src_hf = 0 if rin2 < hh else 1
                cross = src_hf != hf
                r = 0
                while r < rl2:
                    dst0 = ro2 + r
                    take = min(XPC - (dst0 % XPC), rl2 - r)
                    src0 = rin2 + r - hh * src_hf
                    if cross:
                        batch = n_dct - 1
                    else:
                        batch = max(dst0 // XPC, (src0 + take - 1) // XPC)
                    gather_plan[batch].append((hf, dst0, src_hf, src0, take))
                    r += take

    # ---- SE statistic tables (coefficient space) ----
    # recon[r,q] per block = sum_m cm * Rvec[m][r] * Cvec[m][q] with the raw
    # stored coefficients cm = (c00, c01, c02, c10) as the kernel computes
    # them. Duplication weights wr/wc count how often the nearest-resize
    # gather replicates each source row/col.
    wr = np.zeros(h)
    wc = np.zeros(w)
    for i in range(h):
        wr[hi[i]] += 1
    for j in range(w):
        wc[wi[j]] += 1
    Rvec = np.stack([np.ones(8), np.ones(8), np.ones(8), A10 * COS1])
    Cvec = np.stack([A00 * np.ones(8), COS1, COS2, np.ones(8)])
    Tg = h // N
    Rw = np.array(
        [[(wr[8 * t : 8 * t + 8] * Rvec[m]).sum() for t in range(Tg)]
         for m in range(4)]
    )
    Cw = np.array(
        [[(wc[8 * s : 8 * s + 8] * Cvec[m]).sum() for s in range(S)]
         for m in range(4)]
    )
    Wtab = np.einsum("mt,ms->mts", Rw, Cw)  # [4, Tg, S]
    PAIRS = [(0, 0), (0, 1), (0, 2), (0, 3), (1, 1), (1, 2), (1, 3), (2, 2),
             (2, 3), (3, 3)]
    Vtab = []
    for m, n in PAIRS:
        fac = 1.0 if m == n else 2.0
        Rmn = np.array(
            [(wr[8 * t : 8 * t + 8] * Rvec[m] * Rvec[n]).sum() for t in range(Tg)]
        )
        Cmn = np.array(
            [(wc[8 * s : 8 * s + 8] * Cvec[m] * Cvec[n]).sum() for s in range(S)]
        )
        Vtab.append(fac * np.einsum("t,s->ts", Rmn, Cmn))
    Vtab = np.stack(Vtab)  # [10, Tg, S]
    # per-partition: halves see their own block-row range
    Wp = np.zeros((128, 4, T, S), np.float32)
    Vp = np.zeros((128, 10, T, S), np.float32)
    Wp[0:64] = Wtab[None, :, 0:T, :]
    Wp[64:128] = Wtab[None, :, T : 2 * T, :]
    Vp[0:64] = Vtab[None, :, 0:T, :]
    Vp[64:128] = Vtab[None, :, T : 2 * T, :]
    NLIN = 4 * T * S
    NQUAD = 10 * T * S
    tbl_np = np.concatenate([Wp.reshape(128, -1), Vp.reshape(128, -1)], axis=1)

    nc = bacc.Bacc("TRN2")

    x = nc.dram_tensor("x", [C, H, W], FP32, kind="ExternalInput")
    w1 = nc.dram_tensor("w1", [C2, C, 3, 3], FP32, kind="ExternalInput")
    b1 = nc.dram_tensor("b1", [C2], FP32, kind="ExternalInput")
    w2 = nc.dram_tensor("w2", [C2, C2, 3, 3], FP32, kind="ExternalInput")
    b2 = nc.dram_tensor("b2", [C2], FP32, kind="ExternalInput")
    attw = nc.dram_tensor("att_conv_w", [C, C, 1, 1], FP32, kind="ExternalInput")
    attb = nc.dram_tensor("att_conv_b", [C], FP32, kind="ExternalInput")
    fc1 = nc.dram_tensor("fc1_w", [C // 16, C, 1, 1], FP32, kind="ExternalInput")
    fc2 = nc.dram_tensor("fc2_w", [C, C // 16, 1, 1], FP32, kind="ExternalInput")
    out = nc.dram_tensor("out", [C2, h, w], FP32, kind="ExternalOutput")

    const_np = np.zeros((128, 4, 8), np.float32)
    const_np[:, 0, :] = COS1
    const_np[:, 1, :] = COS2
    const_np[:, 2, :] = COS1 * A01
    const_np[:, 3, :] = COS2 * A02
    cdram = nc.inline_tensor(const_np.reshape(128, 32), name="dctconst")
    tdram = nc.inline_tensor(tbl_np, name="stattbl")

    NF = h * w  # pixels per full channel image

    with tile.TileContext(nc) as tc, ExitStack() as ctx:
        wpool = ctx.enter_context(tc.tile_pool(name="wpool", bufs=1))
        smallD = ctx.enter_context(tc.tile_pool(name="smallD", bufs=1))
        small = ctx.enter_context(tc.tile_pool(name="small", bufs=2))
        psA = ctx.enter_context(tc.tile_pool(name="psA", bufs=2, space="PSUM"))
        psC = ctx.enter_context(tc.tile_pool(name="psC", bufs=3, space="PSUM"))
        pin = tc.alloc_tile_pool(name="pin", bufs=2, side="right")
        pws = tc.alloc_tile_pool(name="pws", bufs=1)  # weight staging

        # ---------------- constants / weights ----------------
        consts = wpool.tile([128, 4, 8], FP32)
        nc.sync.dma_start(consts[:], cdram[:].rearrange("p (a b) -> p a b", a=4))
        constsb = wpool.tile([128, 4, 8], BF16)
        nc.vector.tensor_copy(constsb[:], consts[:])

        def cvec(row, shp):  # broadcast [128,8] bf16 const row to shp
            return constsb[:, row, None, None, :].to_broadcast(shp)

        tblb = wpool.tile([128, NLIN + NQUAD], BF16)
        tbls = pws.tile([128, NLIN + NQUAD], FP32)
        nc.sync.dma_start(tbls[:], tdram[:])
        nc.vector.tensor_copy(tblb[:], tbls[:])
        wv = tblb[:, 0:NLIN].rearrange("p (m t s) -> p m t s", m=4, t=T)
        vv = tblb[:, NLIN : NLIN + NQUAD].rearrange(
            "p (m t s) -> p m t s", m=10, t=T
        )

        from concourse.masks import make_identity

        ident = wpool.tile([128, 128], FP32)
        make_identity(nc, ident[:])

        zerot = wpool.tile([128, 1], FP32)
        nc.vector.memset(zerot[:], 0.0)

        w1s = pws.tile([C2, C * 9], FP32)
        nc.sync.dma_start(w1s[:], w1[:].rearrange("o i ky kx -> o (i ky kx)"))
        w1t = wpool.tile([128, 9, C2], BF16)
        for tap in range(9):
            pt = psA.tile([C, C2], FP32, tag="ps")
            sv = w1s[:].rearrange("o (i t) -> o t i", t=9)[:, tap, :]
            nc.tensor.transpose(pt[:], sv, ident[:])
            nc.vector.tensor_copy(w1t[0:CH, tap, :], pt[:])
            nc.vector.tensor_copy(w1t[CH:128, tap, :], pt[:])

        w2s = pws.tile([C2, C2 * 9], FP32)
        nc.sync.dma_start(w2s[:], w2[:].rearrange("o i ky kx -> o (i ky kx)"))
        w2t = wpool.tile([128, 9, C2], BF16)
        for tap in range(9):
            pt = psA.tile([C2, C2], FP32, tag="ps")
            sv = w2s[:].rearrange("o (i t) -> o t i", t=9)[:, tap, :]
            nc.tensor.transpose(pt[:], sv, ident[:])
            nc.vector.tensor_copy(w2t[:, tap, :], pt[:])

        atts = pws.tile([C, C], FP32)
        nc.sync.dma_start(atts[:], attw[:, :, 0, 0])
        attt = wpool.tile([128, C], BF16)
        pt = psA.tile([C, C], FP32, tag="ps")
        nc.tensor.transpose(pt[:], atts[:], ident[0:C, 0:C])
        nc.vector.tensor_copy(attt[0:CH, :], pt[:])
        nc.vector.tensor_copy(attt[CH:128, :], pt[:])

        fc1t = pws.tile([C, C // 16], FP32)
        nc.sync.dma_start(fc1t[:], fc1[:, :, 0, 0].rearrange("o c -> c o"))
        fc1b = wpool.tile([C, C // 16], BF16)
        nc.vector.tensor_copy(fc1b[:], fc1t[:])
        fc2t = pws.tile([C // 16, C], FP32)
        nc.sync.dma_start(fc2t[:], fc2[:, :, 0, 0].rearrange("o c -> c o"))
        fc2b = wpool.tile([C // 16, C], BF16)
        nc.vector.tensor_copy(fc2b[:], fc2t[:])

        b1t = wpool.tile([C2, 1], FP32)
        nc.sync.dma_start(b1t[:], b1[:, None])
        b2t = wpool.tile([C2, 1], FP32)
        nc.sync.dma_start(b2t[:], b2[:, None])
        attbt = wpool.tile([C, 1], FP32)
        nc.sync.dma_start(attbt[:], attb[:, None])
        attg = wpool.tile([128, C], BF16)  # gamma-folded att weights

        pws.release()

        pxa = tc.alloc_tile_pool(name="pxa", bufs=1)
        prec = tc.alloc_tile_pool(name="prec", bufs=1)

        x_all = pxa.tile([128, hh + 2, w + 2], BF16)
        nc.gpsimd.memset(x_all[:, :, 0], 0.0)
        nc.gpsimd.memset(x_all[:, :, w + 1], 0.0)
        nc.gpsimd.memset(x_all[0:CH, 0, :], 0.0)
        nc.gpsimd.memset(x_all[CH:128, hh + 1, :], 0.0)

        recon = prec.tile([128, hh, w], BF16)
        CF = prec.tile([128, 4, T, S], BF16)  # c00, c01, c02, c10
        lin_t = small.tile([128, n_dct], FP32, tag="lin", bufs=1)
        quad_t = small.tile([128, n_dct], FP32, tag="quad", bufs=1)

        lowp = nc.allow_low_precision(reason="bf16 DCT partials, ample tolerance")
        lowp.__enter__()

        # ---------------- phase A: coefficient + stat ops per chunk --------
        def coef_ops(c):
            xa = x_all[:, 1 + c * XPC : 1 + (c + 1) * XPC, 1 : w + 1]
            tsl = slice(c * DCT_T, (c + 1) * DCT_T)
            st = {}
            ops = []

            def f_a0():
                xv = xa.rearrange("p (t r) q -> p t r q", r=N)
                t1 = smallD.tile([128, DCT_T, 4, w], BF16, tag="t1", name="t1")
                nc.vector.tensor_tensor(
                    t1[:], xv[:, :, 0:4, :], xv[:, :, 4:8, :], OP.add
                )
                t2 = smallD.tile([128, DCT_T, 2, w], BF16, tag="t2", name="t2")
                nc.vector.tensor_tensor(
                    t2[:], t1[:, :, 0:2, :], t1[:, :, 2:4, :], OP.add
                )
                a0 = smallD.tile([128, DCT_T, w], BF16, tag="a0", name="a0")
                nc.vector.tensor_tensor(
                    a0[:], t2[:, :, 0, :], t2[:, :, 1, :], OP.add
                )
                st["a0"] = a0

            def f_qs():
                xq = xa.rearrange("p tr (s h4) -> p tr s h4", h4=N)
                q1 = smallD.tile([128, XPC, S, 4], BF16, tag="q1", name="q1")
                nc.vector.tensor_tensor(
                    q1[:], xq[:, :, :, 0:4], xq[:, :, :, 4:8], OP.add
                )
                q2 = smallD.tile([128, XPC, S, 2], BF16, tag="q2", name="q2")
                nc.vector.tensor_tensor(
                    q2[:], q1[:, :, :, 0:2], q1[:, :, :, 2:4], OP.add
                )
                qs = smallD.tile([128, XPC, S], BF16, tag="qs", name="qs")
                nc.vector.tensor_tensor(
                    qs[:], q2[:, :, :, 0], q2[:, :, :, 1], OP.add
                )
                st["qs"] = qs

            def f_c10():
                qsv = st["qs"][:].rearrange("p (t r) s -> p t r s", r=N)
                c10a = smallD.tile([128, DCT_T, S], FP32, tag="c10a", name="c10a")
                nc.vector.tensor_scalar(
                    c10a[:], qsv[:, :, 0, :], float(COS1[0]), None, OP.mult
                )
                for r in range(1, N - 1):
                    nc.vector.scalar_tensor_tensor(
                        c10a[:], qsv[:, :, r, :], float(COS1[r]), c10a[:],
                        OP.mult, OP.add,
                    )
                nc.vector.scalar_tensor_tensor(
                    CF[:, 3, tsl, :], qsv[:, :, N - 1, :], float(COS1[N - 1]),
                    c10a[:], OP.mult, OP.add,
                )

            def f_c0x():
                a0v = st["a0"][:].rearrange("p t (s q) -> p t s q", q=N)
                shp = (128, DCT_T, S, N)
                nc.vector.tensor_reduce(
                    CF[:, 0, tsl, :], a0v, axis=AX.X, op=OP.add
                )
                tmp = smallD.tile([128, DCT_T, w], BF16, tag="tmp", name="tmp")
                tmpv = tmp[:].rearrange("p t (s q) -> p t s q", q=N)
                nc.vector.tensor_tensor(tmpv, a0v, cvec(2, shp), OP.mult)
                nc.vector.tensor_reduce(
                    CF[:, 1, tsl, :], tmpv, axis=AX.X, op=OP.add
                )
                nc.vector.tensor_tensor(tmpv, a0v, cvec(3, shp), OP.mult)
                nc.vector.tensor_reduce(
                    CF[:, 2, tsl, :], tmpv, axis=AX.X, op=OP.add
                )

            def f_lin():
                ld = smallD.tile([128, 4, DCT_T * S], BF16, tag="ld", name="ld")
                nc.vector.tensor_tensor(
                    ld[:],
                    CF[:, :, tsl, :].rearrange("p m t s -> p m (t s)"),
                    wv[:, :, tsl, :].rearrange("p m t s -> p m (t s)"),
                    OP.mult,
                )
                nc.vector.tensor_reduce(
                    lin_t[:, c : c + 1], ld[:, None, :, :], axis=AX.XY, op=OP.add
                )

            def f_quad():
                pp = smallD.tile([128, 10, DCT_T, S], BF16, tag="pp", name="pp")
                for i, (m, n) in enumerate(PAIRS):
                    nc.gpsimd.tensor_tensor(
                        pp[:, i], CF[:, m, tsl, :], CF[:, n, tsl, :], OP.mult
                    )
                qd = smallD.tile([128, 10, DCT_T * S], BF16, tag="qd", name="qd")
                nc.vector.tensor_tensor(
                    qd[:],
                    pp[:].rearrange("p m t s -> p m (t s)"),
                    vv[:, :, tsl, :].rearrange("p m t s -> p m (t s)"),
                    OP.mult,
                )
                nc.vector.tensor_reduce(
                    quad_t[:, c : c + 1], qd[:, None, :, :], axis=AX.XY, op=OP.add
                )

            return [f_a0, f_qs, f_c10, f_c0x, f_lin, f_quad]

        # ---------------- load + maxpool (into x_all) ----------------------
        pending = deque()

        def drain(k):
            for _ in range(k):
                if not pending:
                    return
                pending.popleft()()

        for k in range(n_mp):
            xin = pin.tile([128, 2 * MP_CH, W], FP32, tag="xin", name="xin")
            r0 = 2 * MP_CH * k
            nc.sync.dma_start(xin[0:CH, :, :], x[:, r0 : r0 + 2 * MP_CH, :])
            nc.sync.dma_start(
                xin[CH:128, :, :], x[:, H // 2 + r0 : H // 2 + r0 + 2 * MP_CH, :]
            )
            hmax = pin.tile([128, 2 * MP_CH, w], BF16, tag="hmax", name="hmax",
                            bufs=1)
            xv = xin[:].rearrange("p r (a two) -> p r a two", two=2)
            nc.vector.tensor_tensor(hmax[:], xv[:, :, :, 0], xv[:, :, :, 1],
                                    OP.max)
            hv = hmax[:].rearrange("p (b two) q -> p b two q", two=2)
            nc.vector.tensor_tensor(
                x_all[:, 1 + MP_CH * k : 1 + MP_CH * (k + 1), 1 : w + 1],
                hv[:, :, 0, :], hv[:, :, 1, :], OP.max,
            )
            drain(2)
            if (k + 1) % mp_per_xpc == 0:
                pending.extend(coef_ops(k // mp_per_xpc))
        drain(len(pending) + 1)

        pin.release()

        # ---------------- SE -> gamma ----------------
        ysum = small.tile([128, 1], FP32, tag="ysumT", bufs=1)
        ysq = small.tile([128, 1], FP32, tag="ysqT", bufs=1)
        nc.vector.tensor_reduce(ysum[:], lin_t[:, None, :], axis=AX.X, op=OP.add)
        nc.vector.tensor_reduce(ysq[:], quad_t[:, None, :], axis=AX.X, op=OP.add)

        sestat = small.tile([64, 12], FP32, tag="se", bufs=1)
        yhi = small.tile([64, 2], FP32, tag="yhi", bufs=1)
        nc.sync.dma_start(yhi[:, 0:1], ysum[CH:128, :])
        nc.sync.dma_start(yhi[:, 1:2], ysq[CH:128, :])
        nc.vector.tensor_tensor(sestat[:, 0:1], ysum[0:CH, :], yhi[:, 0:1], OP.add)
        nc.vector.tensor_tensor(sestat[:, 1:2], ysq[0:CH, :], yhi[:, 1:2], OP.add)
        nc.vector.tensor_scalar(sestat[:, 2:3], sestat[:, 0:1], 1.0 / NF, None,
                                OP.mult)
        nc.vector.tensor_scalar(sestat[:, 3:4], sestat[:, 1:2], 1.0 / NF, None,
                                OP.mult)
        nc.vector.tensor_tensor(sestat[:, 4:5], sestat[:, 2:3], sestat[:, 2:3],
                                OP.mult)
        nc.vector.tensor_tensor(sestat[:, 5:6], sestat[:, 3:4], sestat[:, 4:5],
                                OP.subtract)
        nc.vector.tensor_scalar(
            sestat[:, 6:7], sestat[:, 5:6], float(NF) / float(NF - 1), None,
            OP.mult,
        )
        nc.vector.tensor_tensor(sestat[:, 7:8], sestat[:, 2:3], sestat[:, 6:7],
                                OP.add)
        sb = small.tile([64, 1], BF16, tag="sb16", bufs=1)
        nc.vector.tensor_copy(sb[:], sestat[:, 7:8])
        pfc1 = psA.tile([C // 16, 1], FP32, tag="ps")
        nc.tensor.matmul(pfc1[:], fc1b[:], sb[:], start=True, stop=True)
        tb = small.tile([C // 16, 1], BF16, tag="tb16", bufs=1)
        nc.scalar.activation(tb[:], pfc1[:], ACT.Relu)
        pfc2 = psA.tile([C, 1], FP32, tag="ps")
        nc.tensor.matmul(pfc2[:], fc2b[:], tb[:], start=True, stop=True)
        gamma = small.tile([64, 1], FP32, tag="gamma", bufs=1)
        nc.scalar.activation(gamma[:], pfc2[:], ACT.Sigmoid)
        gamma128 = small.tile([128, 1], FP32, tag="g128", bufs=1)
        nc.vector.tensor_copy(gamma128[0:CH, :], gamma[:])
        nc.sync.dma_start(gamma128[CH:128, :], gamma[:])
        nc.vector.tensor_scalar(attg[:], attt[:], gamma128[:, 0:1], None, OP.mult)

        # ---------------- phase B ----------------
        po1 = tc.alloc_tile_pool(name="po1", bufs=1)
        o1 = po1.tile([C2, h + 2, w + 2], BF16)
        nc.gpsimd.memset(o1[:, 0, :], 0.0)
        nc.gpsimd.memset(o1[:, h + 1, :], 0.0)
        nc.gpsimd.memset(o1[:, :, 0], 0.0)
        nc.gpsimd.memset(o1[:, :, w + 1], 0.0)

        ATT_G = 8
        n_fl = (ATT_G * w) // 512
        AN = 512

        # conv1 readiness: per half, the batch after which each x_all tile row
        # is final (gather-subtract applied). 99 = needs halo bounce.
        ready = {0: np.full(hh + 2, -1, int), 1: np.full(hh + 2, -1, int)}
        for batch, entries in gather_plan.items():
            for hf, dst0, src_hf, src0, take in entries:
                ready[hf][1 + dst0 : 1 + dst0 + take] = batch
        ready[0][0] = -1
        ready[1][hh + 1] = -1
        ready[0][hh + 1] = 99
        ready[1][0] = 99
        c1_ready = {
            hf: [int(ready[hf][2 * g : 2 * g + 5].max()) for g in range(n_c1)]
            for hf in (0, 1)
        }

        emitted = {0: set(), 1: set()}
        availq = {0: deque(), 1: deque()}
        queued = {0: set(), 1: set()}
        conv2_left = deque(range(h // RT))
        eng_flip = [0]

        def emit_c1_pair(ga, gb):
            pcA = psC.tile([C2, RT * w], FP32, tag="ps0", name="pcA")
            pcB = psC.tile([C2, RT * w], FP32, tag="ps1", name="pcB")
            la, lb = ga * RT, gb * RT
            for tap in range(9):
                dy, dx = divmod(tap, 3)
                nc.tensor.matmul(
                    pcA[:], w1t[0:CH, tap, :],
                    x_all[0:CH, la + dy : la + dy + RT, dx : dx + w],
                    start=(tap == 0), stop=(tap == 8),
                )
                nc.tensor.matmul(
                    pcB[:], w1t[CH:128, tap, :],
                    x_all[CH:128, lb + dy : lb + dy + RT, dx : dx + w],
                    start=(tap == 0), stop=(tap == 8),
                )
            dstA = o1[:, 1 + la : 1 + la + RT, 1 : w + 1]
            nc.scalar.activation(dstA, pcA[:], ACT.Relu, bias=b1t[:, 0:1])
            dstB = o1[:, 1 + hh + lb : 1 + hh + lb + RT, 1 : w + 1]
            nc.vector.scalar_tensor_tensor(
                dstB, pcB[:], b1t[:, 0:1],
                zerot[:, 0:1, None].to_broadcast((C2, RT, w)),
                OP.add, OP.max,
            )
            emitted[0].add(ga)
            emitted[1].add(gb)

        def emit_c2(g):
            pc = psC.tile([C2, RT * w], FP32,
                          tag=("ps0" if g % 2 == 0 else "ps1"), name="pc2")
            lr = g * RT
            for tap in range(9):
                dy, dx = divmod(tap, 3)
                rhs = o1[:, lr + dy : lr + dy + RT, dx : dx + w]
                nc.tensor.matmul(
                    pc[:], w2t[:, tap, :], rhs, start=(tap == 0), stop=(tap == 8)
                )
            stg = small.tile([C2, RT * w], FP32, tag="ostg", name="ostg")
            if g % 2 == 0:
                nc.scalar.activation(stg[:], pc[:], ACT.Relu, bias=b2t[:, 0:1])
            else:
                nc.vector.scalar_tensor_tensor(
                    stg[:], pc[:], b2t[:, 0:1],
                    zerot[:, 0:1].to_broadcast((C2, RT * w)),
                    OP.add, OP.max,
                )
            nc.sync.dma_start(out[:, lr : lr + RT, :], stg[:])

        def c2_ready(g):
            for R in range(2 * g, 2 * g + 4):
                if R == 0 or R == h + 1:
                    continue
                if R <= hh:
                    if (R - 1) // 2 not in emitted[0]:
                        return False
                else:
                    if (R - 1 - hh) // 2 not in emitted[1]:
                        return False
            return True

        def try_emit(batch_done):
            for hf in (0, 1):
                for g in range(n_c1):
                    if g not in queued[hf] and c1_ready[hf][g] <= batch_done:
                        queued[hf].add(g)
                        availq[hf].append(g)
            while availq[0] and availq[1]:
                emit_c1_pair(availq[0].popleft(), availq[1].popleft())
            while conv2_left and c2_ready(conv2_left[0]):
                emit_c2(conv2_left.popleft())

        for c in range(n_dct):
            tsl = slice(c * DCT_T, (c + 1) * DCT_T)
            shp = (128, DCT_T, S, N)
            # e0 (horizontal reconstruction term) + c10e broadcast
            e0 = smallD.tile([128, DCT_T, w], BF16, tag="e0", name="e0", bufs=2)
            e0v = e0[:].rearrange("p t (s q) -> p t s q", q=N)
            tmp8 = smallD.tile([128, DCT_T, w], BF16, tag="tmp8", name="tmp8")
            tmp8v = tmp8[:].rearrange("p t (s q) -> p t s q", q=N)
            c01b = CF[:, 1, tsl, :, None].to_broadcast(shp)
            c02b = CF[:, 2, tsl, :, None].to_broadcast(shp)
            c00b = CF[:, 0, tsl, :, None].to_broadcast(shp)
            nc.vector.tensor_tensor(e0v, c01b, cvec(0, shp), OP.mult)
            nc.vector.tensor_tensor(tmp8v, c02b, cvec(1, shp), OP.mult)
            nc.vector.tensor_tensor(e0[:], e0[:], tmp8[:], OP.add)
            nc.vector.scalar_tensor_tensor(e0v, c00b, A00, e0v, OP.mult, OP.add)
            c10e = smallD.tile([128, DCT_T, w], BF16, tag="c10e", name="c10e",
                               bufs=2)
            c10ev = c10e[:].rearrange("p t (s q) -> p t s q", q=N)
            nc.scalar.copy(c10ev, CF[:, 3, tsl, :, None].to_broadcast(shp))
            # recon rows
            rv = recon[:, c * XPC : (c + 1) * XPC, :].rearrange(
                "p (t r) q -> p t r q", r=N
            )
            for r in range(N):
                nc.vector.scalar_tensor_tensor(
                    rv[:, :, r, :], c10e[:], float(A10 * COS1[r]), e0[:],
                    OP.mult, OP.add,
                )
            # att 1x1 conv on recon (gamma folded into attg), halves paired
            xcr = small.tile([128, XPC, w], BF16, tag="xcr", name="xcr")
            for sub in range(XPC // ATT_G):
                base = (sub * ATT_G) * w
                r0v = recon[0:CH, c * XPC : (c + 1) * XPC, :].rearrange(
                    "p a b -> p (a b)"
                )
                r1v = recon[CH:128, c * XPC : (c + 1) * XPC, :].rearrange(
                    "p a b -> p (a b)"
                )
                x0v = xcr[0:CH, :, :].rearrange("p a b -> p (a b)")
                x1v = xcr[CH:128, :, :].rearrange("p a b -> p (a b)")
                for f in range(n_fl):
                    sl = slice(base + f * AN, base + (f + 1) * AN)
                    paA = psC.tile([CH, AN], FP32, tag="ps0", name="paA")
                    paB = psC.tile([CH, AN], FP32, tag="ps1", name="paB")
                    nc.tensor.matmul(paA[:], attg[0:CH, :], r0v[:, sl],
                                     start=True, stop=True)
                    nc.tensor.matmul(paB[:], attg[CH:128, :], r1v[:, sl],
                                     start=True, stop=True)
                    nc.scalar.activation(x0v[:, sl], paA[:], ACT.Relu,
                                         bias=attbt[:, 0:1])
                    nc.scalar.activation(x1v[:, sl], paB[:], ACT.Relu,
                                         bias=attbt[:, 0:1])
            # D = recon - xcr (in place; this is what x_all subtracts)
            nc.vector.tensor_tensor(
                recon[:, c * XPC : (c + 1) * XPC, :],
                recon[:, c * XPC : (c + 1) * XPC, :], xcr[:], OP.subtract,
            )
            # gather-subtract batch c: x_all -= D[gathered]
            for hf, dst0, src_hf, src0, take in gather_plan[c]:
                pb = hf * CH
                pbi = src_hf * CH
                if pbi != pb:
                    assert take <= j0
                    xstage = small.tile([128, j0, w], BF16, tag="xstage",
                                        name="xstage")
                    nc.sync.dma_start(
                        xstage[pb : pb + CH, 0:take, :],
                        recon[pbi : pbi + CH, src0 : src0 + take, :],
                    )
                    srct, srow, spb = xstage, 0, pb
                else:
                    srct, srow, spb = recon, src0, pbi
                for co, cin, cl in col_runs:
                    src = srct[spb : spb + CH, srow : srow + take, cin : cin + cl]
                    dst = x_all[pb : pb + CH, 1 + dst0 : 1 + dst0 + take,
                                1 + co : 1 + co + cl]
                    eng = nc.vector if eng_flip[0] % 2 == 0 else nc.gpsimd
                    eng_flip[0] += 1
                    eng.tensor_tensor(dst, dst, src, OP.subtract)
            try_emit(c)

        # cross-half halo rows (x_all fully assembled now)
        nc.sync.dma_start(x_all[CH:128, 0, :], x_all[0:CH, hh, :])
        nc.sync.dma_start(x_all[0:CH, hh + 1, :], x_all[CH:128, 1, :])
        try_emit(99)
        assert not availq[0] and not availq[1] and not conv2_left

        lowp.__exit__(None, None, None)
        po1.release()
        prec.release()
        pxa.release()

    nc.finalize()
    return nc


_NC_CACHE = {}


def _get_nc(H=384, W=384, debug=False):
    key = (H, W, debug)
    if key not in _NC_CACHE:
        _NC_CACHE[key] = build_nc(H=H, W=W, debug=debug)
    return _NC_CACHE[key]


def kernel(x, w1, b1, w2, b2, att_conv_w, att_conv_b, fc1_w, fc2_w):
    x = np.ascontiguousarray(np.asarray(x, np.float32))
    B = x.shape[0]
    nc = _get_nc(x.shape[2], x.shape[3])
    shared = {
        "w1": np.ascontiguousarray(np.asarray(w1, np.float32)),
        "b1": np.ascontiguousarray(np.asarray(b1, np.float32)),
        "w2": np.ascontiguousarray(np.asarray(w2, np.float32)),
        "b2": np.ascontiguousarray(np.asarray(b2, np.float32)),
        "att_conv_w": np.ascontiguousarray(np.asarray(att_conv_w, np.float32)),
        "att_conv_b": np.ascontiguousarray(np.asarray(att_conv_b, np.float32)),
        "fc1_w": np.ascontiguousarray(np.asarray(fc1_w, np.float32)),
        "fc2_w": np.ascontiguousarray(np.asarray(fc2_w, np.float32)),
    }
    in_maps = [dict(shared, x=np.ascontiguousarray(x[i])) for i in range(B)]
    res = run_bass_kernel_spmd(nc, in_maps, core_ids=list(range(B)))
    return np.stack([res.results[i]["out"] for i in range(B)], axis=0)


# revision 23
# speedup vs baseline: 1.1503x; 1.1175x over previous
"""Trainium2 Bass kernel for nn_DownBlock_res_dct1 (maxpool 2x2 + truncated
block-DCT low-pass + SE attention + 1x1 conv + two 3x3 convs), data-parallel
over the batch across 8 NeuronCores.

Self-contained: hardcodes all shapes/constants; builds one SPMD Bass module
(one batch item per core), runs via run_bass_kernel_spmd, gathers the full
(8, 128, 192, 192) output.

Layout: partitions p = half*64 + ch (image row halves). Phase A streams the
input DMA while maxpool (into x_all directly) + DCT coefficients + SE stats
run on DVE/GpSimd. The SE statistics (mean/var of y1) are computed in
COEFFICIENT space: sum(y1) and sum(y1^2) are linear/bilinear functionals of
the 4 kept DCT coefficients with per-block weight tables that fold in the
nearest-resize row/col duplication — so gamma is ready right after the last
coefficient chunk, without materializing y1.

Phase B per 16-row chunk: recon from coefficients, att 1x1 conv computed ON
recon (xcr = relu(attg.recon + b), gamma folded into weights), D = recon -
xcr in place, then x_all -= D[gathered] (fused gather-subtract; x_all already
holds the pooled image). conv1 runs as concurrent K=64 row-tile matmul pairs
(partition halves at PE tile positions (0,0)/(64,0)), conv2 as a K=128
stream, scheduled per-chunk behind the gather so the PE never starves.
"""

import math
from collections import deque
from contextlib import ExitStack

import numpy as np

import concourse.bass as bass
import concourse.mybir as mybir
import concourse.tile as tile
from concourse import bacc
from concourse.bass_utils import run_bass_kernel_spmd

FP32 = mybir.dt.float32
BF16 = mybir.dt.bfloat16
AX = mybir.AxisListType
OP = mybir.AluOpType
ACT = mybir.ActivationFunctionType

N = 8  # DCT block size
_P8 = np.arange(8)
COS1 = np.cos(math.pi * (_P8 + 0.5) / 8.0 * 1).astype(np.float64)
COS2 = np.cos(math.pi * (_P8 + 0.5) / 8.0 * 2).astype(np.float64)
# Selected zigzag coeffs [0,1,2,5] -> (k1,k2) in {(0,0),(0,1),(1,0),(0,2)}
A00 = (1.0 / 8.0) ** 2
A01 = 2.0 / 64.0
A02 = 2.0 / 64.0
A10 = 2.0 / 64.0


def _runs(idx):
    """Contiguous runs where idx[i] = i - g: list of (out_start, in_start, len)."""
    runs = []
    s = 0
    for i in range(1, len(idx) + 1):
        if i == len(idx) or idx[i] != idx[i - 1] + 1:
            runs.append((s, int(idx[s]), i - s))
            s = i
    return runs


def _split_at(ro, rin, rl, bound):
    """Split a run at source-row `bound`."""
    if rin < bound < rin + rl:
        return [(ro, rin, bound - rin), (ro + bound - rin, bound, rin + rl - bound)]
    return [(ro, rin, rl)]


def build_nc(H=384, W=384, debug=False):
    C, C2 = 64, 128
    h, w = H // 2, W // 2
    hh = h // 2  # rows per half
    assert hh % N == 0 and w % N == 0
    T = hh // N  # block-rows per half
    S = w // N  # block-cols
    CH = 64

    hi = (np.arange(h) * (h - (N - 1))) // h
    wi = (np.arange(w) * (w - (N - 1))) // w
    col_runs = _runs(wi)
    row_runs_h = [_runs(hi[hh * hf : hh * (hf + 1)]) for hf in (0, 1)]

    MP_CH = 4  # pooled rows per maxpool chunk
    n_mp = hh // MP_CH
    DCT_T = 2  # block-rows per chunk
    n_dct = T // DCT_T
    XPC = DCT_T * N  # pooled rows per chunk
    mp_per_xpc = XPC // MP_CH
    assert XPC % MP_CH == 0
    RT = 2  # conv output rows per matmul group
    n_c1 = hh // RT

    # first half-1 row whose gather source is still in half 0
    j0 = int(np.argmax(hi[hh : 2 * hh] >= hh))
    assert 0 < j0 < XPC

    # Gather plan: (batch, hf, dst_row, src_hf, src_local, len). Batch c means
    # the subtract runs after D-chunk c exists (src rows <= chunk c).
    gather_plan = {c: [] for c in range(n_dct)}
    for hf in (0, 1):
        for ro, rin_g, rl in row_runs_h[hf]:
            for ro2, rin2, rl2 in _split_at(ro, rin_g, rl, hh):
                src_hf = 0 if rin2 < hh else 1
                cross = src_hf != hf
                r = 0
                while r < rl2:
                    dst0 = ro2 + r
                    take = min(XPC - (dst0 % XPC), rl2 - r)
                    src0 = rin2 + r - hh * src_hf
                    if cross:
                        batch = n_dct - 1
                    else:
                        batch = max(dst0 // XPC, (src0 + take - 1) // XPC)
                    gather_plan[batch].append((hf, dst0, src_hf, src0, take))
                    r += take

    # ---- SE statistic tables (coefficient space) ----
    # recon[r,q] per block = sum_m cm * Rvec[m][r] * Cvec[m][q] with the raw
    # stored coefficients cm = (c00, c01, c02, c10) as the kernel computes
    # them. Duplication weights wr/wc count how often the nearest-resize
    # gather replicates each source row/col.
    wr = np.zeros(h)
    wc = np.zeros(w)
    for i in range(h):
        wr[hi[i]] += 1
    for j in range(w):
        wc[wi[j]] += 1
    Rvec = np.stack([np.ones(8), np.ones(8), np.ones(8), A10 * COS1])
    Cvec = np.stack([A00 * np.ones(8), COS1, COS2, np.ones(8)])
    Tg = h // N
    Rw = np.array(
        [[(wr[8 * t : 8 * t + 8] * Rvec[m]).sum() for t in range(Tg)]
         for m in range(4)]
    )
    Cw = np.array(
        [[(wc[8 * s : 8 * s + 8] * Cvec[m]).sum() for s in range(S)]
         for m in range(4)]
    )
    Wtab = np.einsum("mt,ms->mts", Rw, Cw)  # [4, Tg, S]
    PAIRS = [(0, 0), (0, 1), (0, 2), (0, 3), (1, 1), (1, 2), (1, 3), (2, 2),
             (2, 3), (3, 3)]
    Vtab = []
    for m, n in PAIRS:
        fac = 1.0 if m == n else 2.0
        Rmn = np.array(
            [(wr[8 * t : 8 * t + 8] * Rvec[m] * Rvec[n]).sum() for t in range(Tg)]
        )
        Cmn = np.array(
            [(wc[8 * s : 8 * s + 8] * Cvec[m] * Cvec[n]).sum() for s in range(S)]
        )
        Vtab.append(fac * np.einsum("t,s->ts", Rmn, Cmn))
    Vtab = np.stack(Vtab)  # [10, Tg, S]
    # per-partition: halves see their own block-row range
    Wp = np.zeros((128, 4, T, S), np.float32)
    Vp = np.zeros((128, 10, T, S), np.float32)
    Wp[0:64] = Wtab[None, :, 0:T, :]
    Wp[64:128] = Wtab[None, :, T : 2 * T, :]
    Vp[0:64] = Vtab[None, :, 0:T, :]
    Vp[64:128] = Vtab[None, :, T : 2 * T, :]
    NLIN = 4 * T * S
    NQUAD = 10 * T * S
    tbl_np = np.concatenate([Wp.reshape(128, -1), Vp.reshape(128, -1)], axis=1)

    nc = bacc.Bacc("TRN2")

    x = nc.dram_tensor("x", [C, H, W], FP32, kind="ExternalInput")
    w1 = nc.dram_tensor("w1", [C2, C, 3, 3], FP32, kind="ExternalInput")
    b1 = nc.dram_tensor("b1", [C2], FP32, kind="ExternalInput")
    w2 = nc.dram_tensor("w2", [C2, C2, 3, 3], FP32, kind="ExternalInput")
    b2 = nc.dram_tensor("b2", [C2], FP32, kind="ExternalInput")
    attw = nc.dram_tensor("att_conv_w", [C, C, 1, 1], FP32, kind="ExternalInput")
    attb = nc.dram_tensor("att_conv_b", [C], FP32, kind="ExternalInput")
    fc1 = nc.dram_tensor("fc1_w", [C // 16, C, 1, 1], FP32, kind="ExternalInput")
    fc2 = nc.dram_tensor("fc2_w", [C, C // 16, 1, 1], FP32, kind="ExternalInput")
    out = nc.dram_tensor("out", [C2, h, w], FP32, kind="ExternalOutput")

    const_np = np.zeros((128, 4, 8), np.float32)
    const_np[:, 0, :] = COS1
    const_np[:, 1, :] = COS2
    const_np[:, 2, :] = COS1 * A01
    const_np[:, 3, :] = COS2 * A02
    cdram = nc.inline_tensor(const_np.reshape(128, 32), name="dctconst")
    tdram = nc.inline_tensor(tbl_np, name="stattbl")

    NF = h * w  # pixels per full channel image

    with tile.TileContext(nc) as tc, ExitStack() as ctx:
        wpool = ctx.enter_context(tc.tile_pool(name="wpool", bufs=1))
        smallD = ctx.enter_context(tc.tile_pool(name="smallD", bufs=1))
        small = ctx.enter_context(tc.tile_pool(name="small", bufs=2))
        psC = ctx.enter_context(tc.tile_pool(name="psC", bufs=4, space="PSUM"))
        pin = tc.alloc_tile_pool(name="pin", bufs=2, side="right")
        pws = tc.alloc_tile_pool(name="pws", bufs=1)  # weight staging

        # ---------------- constants / weights ----------------
        consts = wpool.tile([128, 4, 8], FP32)
        nc.sync.dma_start(consts[:], cdram[:].rearrange("p (a b) -> p a b", a=4))
        constsb = wpool.tile([128, 4, 8], BF16)
        nc.vector.tensor_copy(constsb[:], consts[:])

        def cvec(row, shp):  # broadcast [128,8] bf16 const row to shp
            return constsb[:, row, None, None, :].to_broadcast(shp)

        tblb = wpool.tile([128, NLIN + NQUAD], BF16)
        tbls = pws.tile([128, NLIN + NQUAD], FP32)
        nc.sync.dma_start(tbls[:], tdram[:])
        nc.vector.tensor_copy(tblb[:], tbls[:])
        wv = tblb[:, 0:NLIN].rearrange("p (m t s) -> p m t s", m=4, t=T)
        vv = tblb[:, NLIN : NLIN + NQUAD].rearrange(
            "p (m t s) -> p m t s", m=10, t=T
        )

        from concourse.masks import make_identity

        ident = wpool.tile([128, 128], FP32)
        make_identity(nc, ident[:])

        zerot = wpool.tile([128, 1], FP32)
        nc.vector.memset(zerot[:], 0.0)

        w1s = pws.tile([C2, C * 9], FP32)
        nc.sync.dma_start(w1s[:], w1[:].rearrange("o i ky kx -> o (i ky kx)"))
        w1t = wpool.tile([128, 9, C2], BF16)
        for tap in range(9):
            pt = psC.tile([C, C2], FP32, tag=("ps0" if tap % 2 == 0 else "ps1"),
                          name="pt")
            sv = w1s[:].rearrange("o (i t) -> o t i", t=9)[:, tap, :]
            nc.tensor.transpose(pt[:], sv, ident[:])
            nc.vector.tensor_copy(w1t[0:CH, tap, :], pt[:])
            nc.vector.tensor_copy(w1t[CH:128, tap, :], pt[:])

        w2s = pws.tile([C2, C2 * 9], FP32)
        nc.sync.dma_start(w2s[:], w2[:].rearrange("o i ky kx -> o (i ky kx)"))
        w2t = wpool.tile([128, 9, C2], BF16)
        for tap in range(9):
            pt = psC.tile([C2, C2], FP32, tag=("ps0" if tap % 2 == 0 else "ps1"),
                          name="pt")
            sv = w2s[:].rearrange("o (i t) -> o t i", t=9)[:, tap, :]
            nc.tensor.transpose(pt[:], sv, ident[:])
            nc.vector.tensor_copy(w2t[:, tap, :], pt[:])

        atts = pws.tile([C, C], FP32)
        nc.sync.dma_start(atts[:], attw[:, :, 0, 0])
        attt = wpool.tile([128, C], BF16)
        pt = psC.tile([C, C], FP32, tag="ps0", name="pt")
        nc.tensor.transpose(pt[:], atts[:], ident[0:C, 0:C])
        nc.vector.tensor_copy(attt[0:CH, :], pt[:])
        nc.vector.tensor_copy(attt[CH:128, :], pt[:])

        fc1t = pws.tile([C, C // 16], FP32)
        nc.sync.dma_start(fc1t[:], fc1[:, :, 0, 0].rearrange("o c -> c o"))
        fc1b = wpool.tile([C, C // 16], BF16)
        nc.vector.tensor_copy(fc1b[:], fc1t[:])
        fc2t = pws.tile([C // 16, C], FP32)
        nc.sync.dma_start(fc2t[:], fc2[:, :, 0, 0].rearrange("o c -> c o"))
        fc2b = wpool.tile([C // 16, C], BF16)
        nc.vector.tensor_copy(fc2b[:], fc2t[:])

        b1t = wpool.tile([C2, 1], FP32)
        nc.sync.dma_start(b1t[:], b1[:, None])
        b2t = wpool.tile([C2, 1], FP32)
        nc.sync.dma_start(b2t[:], b2[:, None])
        attbt = wpool.tile([C, 1], FP32)
        nc.sync.dma_start(attbt[:], attb[:, None])
        attg = wpool.tile([128, C], BF16)  # gamma-folded att weights

        pws.release()

        pxa = tc.alloc_tile_pool(name="pxa", bufs=1)
        prec = tc.alloc_tile_pool(name="prec", bufs=1)

        x_all = pxa.tile([128, hh + 2, w + 2], BF16)
        nc.gpsimd.memset(x_all[:, :, 0], 0.0)
        nc.gpsimd.memset(x_all[:, :, w + 1], 0.0)
        nc.gpsimd.memset(x_all[0:CH, 0, :], 0.0)
        nc.gpsimd.memset(x_all[CH:128, hh + 1, :], 0.0)

        recon = prec.tile([128, hh, w], BF16)
        CF = prec.tile([128, 4, T, S], BF16)  # c00, c01, c02, c10
        lin_t = small.tile([128, n_dct], FP32, tag="lin", bufs=1)
        quad_t = small.tile([128, n_dct], FP32, tag="quad", bufs=1)

        lowp = nc.allow_low_precision(reason="bf16 DCT partials, ample tolerance")
        lowp.__enter__()

        # ---------------- phase A: coefficient + stat ops per chunk --------
        def coef_ops(c):
            xa = x_all[:, 1 + c * XPC : 1 + (c + 1) * XPC, 1 : w + 1]
            tsl = slice(c * DCT_T, (c + 1) * DCT_T)
            st = {}
            ops = []

            def f_a0():
                xv = xa.rearrange("p (t r) q -> p t r q", r=N)
                t1 = smallD.tile([128, DCT_T, 4, w], BF16, tag="t1", name="t1")
                nc.vector.tensor_tensor(
                    t1[:], xv[:, :, 0:4, :], xv[:, :, 4:8, :], OP.add
                )
                t2 = smallD.tile([128, DCT_T, 2, w], BF16, tag="t2", name="t2")
                nc.vector.tensor_tensor(
                    t2[:], t1[:, :, 0:2, :], t1[:, :, 2:4, :], OP.add
                )
                a0 = smallD.tile([128, DCT_T, w], BF16, tag="a0", name="a0")
                nc.vector.tensor_tensor(
                    a0[:], t2[:, :, 0, :], t2[:, :, 1, :], OP.add
                )
                st["a0"] = a0

            def f_qs():
                # col-sums per 8-group (on GpSimd; DVE is the scarce engine)
                xq = xa.rearrange("p tr (s h4) -> p tr s h4", h4=N)
                q1 = smallD.tile([128, XPC, S, 4], BF16, tag="q1", name="q1")
                nc.gpsimd.tensor_tensor(
                    q1[:], xq[:, :, :, 0:4], xq[:, :, :, 4:8], OP.add
                )
                q2 = smallD.tile([128, XPC, S, 2], BF16, tag="q2", name="q2")
                nc.gpsimd.tensor_tensor(
                    q2[:], q1[:, :, :, 0:2], q1[:, :, :, 2:4], OP.add
                )
                qs = smallD.tile([128, XPC, S], BF16, tag="qs", name="qs")
                nc.gpsimd.tensor_tensor(
                    qs[:], q2[:, :, :, 0], q2[:, :, :, 1], OP.add
                )
                st["qs"] = qs

            def f_c10():
                # c10 = sum_r COS1[r]*qs[t,r,s]: weight then tree (GpSimd)
                qsw = smallD.tile([128, XPC, S], BF16, tag="qsw", name="qsw")
                cosb = constsb[:, 0, None, :, None].to_broadcast(
                    (128, DCT_T, N, S)
                )
                qsv = st["qs"][:].rearrange("p (t r) s -> p t r s", r=N)
                nc.gpsimd.tensor_tensor(
                    qsw[:].rearrange("p (t r) s -> p t r s", r=N), qsv, cosb,
                    OP.mult,
                )
                qw = qsw[:].rearrange("p (t r) s -> p t r s", r=N)
                w1_ = smallD.tile([128, DCT_T, 4, S], BF16, tag="w1_", name="w1_")
                nc.gpsimd.tensor_tensor(
                    w1_[:], qw[:, :, 0:4, :], qw[:, :, 4:8, :], OP.add
                )
                w2_ = smallD.tile([128, DCT_T, 2, S], BF16, tag="w2_", name="w2_")
                nc.gpsimd.tensor_tensor(
                    w2_[:], w1_[:, :, 0:2, :], w1_[:, :, 2:4, :], OP.add
                )
                nc.gpsimd.tensor_tensor(
                    CF[:, 3, tsl, :], w2_[:, :, 0, :], w2_[:, :, 1, :], OP.add
                )

            def f_c0x():
                a0v = st["a0"][:].rearrange("p t (s q) -> p t s q", q=N)
                shp = (128, DCT_T, S, N)
                nc.vector.tensor_reduce(
                    CF[:, 0, tsl, :], a0v, axis=AX.X, op=OP.add
                )
                tmp = smallD.tile([128, DCT_T, w], BF16, tag="tmp", name="tmp")
                tmpv = tmp[:].rearrange("p t (s q) -> p t s q", q=N)
                nc.gpsimd.tensor_tensor(tmpv, a0v, cvec(2, shp), OP.mult)
                nc.vector.tensor_reduce(
                    CF[:, 1, tsl, :], tmpv, axis=AX.X, op=OP.add
                )
                tmpb = smallD.tile([128, DCT_T, w], BF16, tag="tmpb", name="tmpb")
                tmpbv = tmpb[:].rearrange("p t (s q) -> p t s q", q=N)
                nc.gpsimd.tensor_tensor(tmpbv, a0v, cvec(3, shp), OP.mult)
                nc.vector.tensor_reduce(
                    CF[:, 2, tsl, :], tmpbv, axis=AX.X, op=OP.add
                )

            def f_lin():
                ld = smallD.tile([128, 4, DCT_T * S], BF16, tag="ld", name="ld")
                nc.vector.tensor_tensor(
                    ld[:],
                    CF[:, :, tsl, :].rearrange("p m t s -> p m (t s)"),
                    wv[:, :, tsl, :].rearrange("p m t s -> p m (t s)"),
                    OP.mult,
                )
                nc.vector.tensor_reduce(
                    lin_t[:, c : c + 1], ld[:, None, :, :], axis=AX.XY, op=OP.add
                )

            def f_quad():
                pp = smallD.tile([128, 10, DCT_T, S], BF16, tag="pp", name="pp")
                for i, (m, n) in enumerate(PAIRS):
                    nc.gpsimd.tensor_tensor(
                        pp[:, i], CF[:, m, tsl, :], CF[:, n, tsl, :], OP.mult
                    )
                qd = smallD.tile([128, 10, DCT_T * S], BF16, tag="qd", name="qd")
                nc.vector.tensor_tensor(
                    qd[:],
                    pp[:].rearrange("p m t s -> p m (t s)"),
                    vv[:, :, tsl, :].rearrange("p m t s -> p m (t s)"),
                    OP.mult,
                )
                nc.vector.tensor_reduce(
                    quad_t[:, c : c + 1], qd[:, None, :, :], axis=AX.XY, op=OP.add
                )

            return [f_a0, f_qs, f_c10, f_c0x, f_lin, f_quad]

        # ---------------- load + maxpool (into x_all) ----------------------
        pending = deque()

        def drain(k):
            for _ in range(k):
                if not pending:
                    return
                pending.popleft()()

        for k in range(n_mp):
            xin = pin.tile([128, 2 * MP_CH, W], FP32, tag="xin", name="xin")
            r0 = 2 * MP_CH * k
            nc.sync.dma_start(xin[0:CH, :, :], x[:, r0 : r0 + 2 * MP_CH, :])
            nc.sync.dma_start(
                xin[CH:128, :, :], x[:, H // 2 + r0 : H // 2 + r0 + 2 * MP_CH, :]
            )
            hmax = pin.tile([128, 2 * MP_CH, w], BF16, tag="hmax", name="hmax",
                            bufs=1)
            xv = xin[:].rearrange("p r (a two) -> p r a two", two=2)
            nc.vector.tensor_tensor(hmax[:], xv[:, :, :, 0], xv[:, :, :, 1],
                                    OP.max)
            hv = hmax[:].rearrange("p (b two) q -> p b two q", two=2)
            nc.vector.tensor_tensor(
                x_all[:, 1 + MP_CH * k : 1 + MP_CH * (k + 1), 1 : w + 1],
                hv[:, :, 0, :], hv[:, :, 1, :], OP.max,
            )
            drain(2)
            if (k + 1) % mp_per_xpc == 0:
                pending.extend(coef_ops(k // mp_per_xpc))
        drain(len(pending) + 1)

        pin.release()

        # ---------------- SE -> gamma ----------------
        ysum = small.tile([128, 1], FP32, tag="ysumT", bufs=1)
        ysq = small.tile([128, 1], FP32, tag="ysqT", bufs=1)
        nc.vector.tensor_reduce(ysum[:], lin_t[:, None, :], axis=AX.X, op=OP.add)
        nc.vector.tensor_reduce(ysq[:], quad_t[:, None, :], axis=AX.X, op=OP.add)

        sestat = small.tile([64, 12], FP32, tag="se", bufs=1)
        yhi = small.tile([64, 2], FP32, tag="yhi", bufs=1)
        nc.sync.dma_start(yhi[:, 0:1], ysum[CH:128, :])
        nc.sync.dma_start(yhi[:, 1:2], ysq[CH:128, :])
        nc.vector.tensor_tensor(sestat[:, 0:1], ysum[0:CH, :], yhi[:, 0:1], OP.add)
        nc.vector.tensor_tensor(sestat[:, 1:2], ysq[0:CH, :], yhi[:, 1:2], OP.add)
        nc.vector.tensor_scalar(sestat[:, 2:3], sestat[:, 0:1], 1.0 / NF, None,
                                OP.mult)
        nc.vector.tensor_scalar(sestat[:, 3:4], sestat[:, 1:2], 1.0 / NF, None,
                                OP.mult)
        nc.vector.tensor_tensor(sestat[:, 4:5], sestat[:, 2:3], sestat[:, 2:3],
                                OP.mult)
        nc.vector.tensor_tensor(sestat[:, 5:6], sestat[:, 3:4], sestat[:, 4:5],
                                OP.subtract)
        nc.vector.tensor_scalar(
            sestat[:, 6:7], sestat[:, 5:6], float(NF) / float(NF - 1), None,
            OP.mult,
        )
        nc.vector.tensor_tensor(sestat[:, 7:8], sestat[:, 2:3], sestat[:, 6:7],
                                OP.add)
        sb = small.tile([64, 1], BF16, tag="sb16", bufs=1)
        nc.vector.tensor_copy(sb[:], sestat[:, 7:8])
        pfc1 = psC.tile([C // 16, 1], FP32, tag="ps0", name="pfc1")
        nc.tensor.matmul(pfc1[:], fc1b[:], sb[:], start=True, stop=True)
        tb = small.tile([C // 16, 1], BF16, tag="tb16", bufs=1)
        nc.scalar.activation(tb[:], pfc1[:], ACT.Relu)
        pfc2 = psC.tile([C, 1], FP32, tag="ps1", name="pfc2")
        nc.tensor.matmul(pfc2[:], fc2b[:], tb[:], start=True, stop=True)
        gamma = small.tile([64, 1], FP32, tag="gamma", bufs=1)
        nc.scalar.activation(gamma[:], pfc2[:], ACT.Sigmoid)
        gamma128 = small.tile([128, 1], FP32, tag="g128", bufs=1)
        nc.vector.tensor_copy(gamma128[0:CH, :], gamma[:])
        nc.sync.dma_start(gamma128[CH:128, :], gamma[:])
        nc.vector.tensor_scalar(attg[:], attt[:], gamma128[:, 0:1], None, OP.mult)

        # ---------------- phase B ----------------
        po1 = tc.alloc_tile_pool(name="po1", bufs=1)
        o1 = po1.tile([C2, h + 2, w + 2], BF16)
        nc.gpsimd.memset(o1[:, 0, :], 0.0)
        nc.gpsimd.memset(o1[:, h + 1, :], 0.0)
        nc.gpsimd.memset(o1[:, :, 0], 0.0)
        nc.gpsimd.memset(o1[:, :, w + 1], 0.0)

        ATT_G = 8
        n_fl = (ATT_G * w) // 512
        AN = 512

        # conv1 readiness: per half, the batch after which each x_all tile row
        # is final (gather-subtract applied). 99 = needs halo bounce.
        ready = {0: np.full(hh + 2, -1, int), 1: np.full(hh + 2, -1, int)}
        for batch, entries in gather_plan.items():
            for hf, dst0, src_hf, src0, take in entries:
                ready[hf][1 + dst0 : 1 + dst0 + take] = batch
        ready[0][0] = -1
        ready[1][hh + 1] = -1
        ready[0][hh + 1] = 99
        ready[1][0] = 99
        c1_ready = {
            hf: [int(ready[hf][2 * g : 2 * g + 5].max()) for g in range(n_c1)]
            for hf in (0, 1)
        }

        emitted = {0: set(), 1: set()}
        availq = {0: deque(), 1: deque()}
        queued = {0: set(), 1: set()}
        conv2_left = deque(range(h // RT))
        eng_flip = [0]

        def emit_c1_pair(ga, gb):
            pcA = psC.tile([C2, RT * w], FP32, tag="ps0", name="pcA")
            pcB = psC.tile([C2, RT * w], FP32, tag="ps1", name="pcB")
            la, lb = ga * RT, gb * RT
            for tap in range(9):
                dy, dx = divmod(tap, 3)
                nc.tensor.matmul(
                    pcA[:], w1t[0:CH, tap, :],
                    x_all[0:CH, la + dy : la + dy + RT, dx : dx + w],
                    start=(tap == 0), stop=(tap == 8),
                )
                nc.tensor.matmul(
                    pcB[:], w1t[CH:128, tap, :],
                    x_all[CH:128, lb + dy : lb + dy + RT, dx : dx + w],
                    start=(tap == 0), stop=(tap == 8),
                )
            dstA = o1[:, 1 + la : 1 + la + RT, 1 : w + 1]
            nc.scalar.activation(dstA, pcA[:], ACT.Relu, bias=b1t[:, 0:1])
            dstB = o1[:, 1 + hh + lb : 1 + hh + lb + RT, 1 : w + 1]
            nc.vector.scalar_tensor_tensor(
                dstB, pcB[:], b1t[:, 0:1],
                zerot[:, 0:1, None].to_broadcast((C2, RT, w)),
                OP.add, OP.max,
            )
            emitted[0].add(ga)
            emitted[1].add(gb)

        def emit_c2(g):
            pc = psC.tile([C2, RT * w], FP32,
                          tag=("ps0" if g % 2 == 0 else "ps1"), name="pc2")
            lr = g * RT
            for tap in range(9):
                dy, dx = divmod(tap, 3)
                rhs = o1[:, lr + dy : lr + dy + RT, dx : dx + w]
                nc.tensor.matmul(
                    pc[:], w2t[:, tap, :], rhs, start=(tap == 0), stop=(tap == 8)
                )
            stg = small.tile([C2, RT * w], FP32, tag="ostg", name="ostg")
            if g % 2 == 0:
                nc.scalar.activation(stg[:], pc[:], ACT.Relu, bias=b2t[:, 0:1])
            else:
                nc.vector.scalar_tensor_tensor(
                    stg[:], pc[:], b2t[:, 0:1],
                    zerot[:, 0:1].to_broadcast((C2, RT * w)),
                    OP.add, OP.max,
                )
            nc.sync.dma_start(out[:, lr : lr + RT, :], stg[:])

        def c2_ready(g):
            for R in range(2 * g, 2 * g + 4):
                if R == 0 or R == h + 1:
                    continue
                if R <= hh:
                    if (R - 1) // 2 not in emitted[0]:
                        return False
                else:
                    if (R - 1 - hh) // 2 not in emitted[1]:
                        return False
            return True

        def try_emit(batch_done):
            for hf in (0, 1):
                for g in range(n_c1):
                    if g not in queued[hf] and c1_ready[hf][g] <= batch_done:
                        queued[hf].add(g)
                        availq[hf].append(g)
            while availq[0] and availq[1]:
                emit_c1_pair(availq[0].popleft(), availq[1].popleft())
            while conv2_left and c2_ready(conv2_left[0]):
                emit_c2(conv2_left.popleft())

        def emit_recon(c):
            tsl = slice(c * DCT_T, (c + 1) * DCT_T)
            shp = (128, DCT_T, S, N)
            # e0 (horizontal reconstruction term) + c10e broadcast
            e0 = smallD.tile([128, DCT_T, w], BF16, tag="e0", name="e0", bufs=2)
            e0v = e0[:].rearrange("p t (s q) -> p t s q", q=N)
            tmp8 = smallD.tile([128, DCT_T, w], BF16, tag="tmp8", name="tmp8")
            tmp8v = tmp8[:].rearrange("p t (s q) -> p t s q", q=N)
            c01b = CF[:, 1, tsl, :, None].to_broadcast(shp)
            c02b = CF[:, 2, tsl, :, None].to_broadcast(shp)
            c00b = CF[:, 0, tsl, :, None].to_broadcast(shp)
            nc.vector.tensor_tensor(e0v, c01b, cvec(0, shp), OP.mult)
            nc.vector.tensor_tensor(tmp8v, c02b, cvec(1, shp), OP.mult)
            nc.vector.tensor_tensor(e0[:], e0[:], tmp8[:], OP.add)
            nc.vector.scalar_tensor_tensor(e0v, c00b, A00, e0v, OP.mult, OP.add)
            c10e = smallD.tile([128, DCT_T, w], BF16, tag="c10e", name="c10e",
                               bufs=2)
            c10ev = c10e[:].rearrange("p t (s q) -> p t s q", q=N)
            nc.scalar.copy(c10ev, CF[:, 3, tsl, :, None].to_broadcast(shp))
            # recon rows
            rv = recon[:, c * XPC : (c + 1) * XPC, :].rearrange(
                "p (t r) q -> p t r q", r=N
            )
            for r in range(N):
                nc.vector.scalar_tensor_tensor(
                    rv[:, :, r, :], c10e[:], float(A10 * COS1[r]), e0[:],
                    OP.mult, OP.add,
                )

        # chunks 0-1 reconstructed during gamma's fc/sigmoid latency
        emit_recon(0)
        emit_recon(1)

        for c in range(n_dct):
            tsl = slice(c * DCT_T, (c + 1) * DCT_T)
            shp = (128, DCT_T, S, N)
            if c >= 2:
                emit_recon(c)
            # att 1x1 conv on recon (gamma folded into attg), halves paired
            xcr = small.tile([128, XPC, w], BF16, tag="xcr", name="xcr")
            for sub in range(XPC // ATT_G):
                base = (sub * ATT_G) * w
                r0v = recon[0:CH, c * XPC : (c + 1) * XPC, :].rearrange(
                    "p a b -> p (a b)"
                )
                r1v = recon[CH:128, c * XPC : (c + 1) * XPC, :].rearrange(
                    "p a b -> p (a b)"
                )
                x0v = xcr[0:CH, :, :].rearrange("p a b -> p (a b)")
                x1v = xcr[CH:128, :, :].rearrange("p a b -> p (a b)")
                for f in range(n_fl):
                    sl = slice(base + f * AN, base + (f + 1) * AN)
                    paA = psC.tile([CH, AN], FP32, tag="ps0", name="paA")
                    paB = psC.tile([CH, AN], FP32, tag="ps1", name="paB")
                    nc.tensor.matmul(paA[:], attg[0:CH, :], r0v[:, sl],
                                     start=True, stop=True)
                    nc.tensor.matmul(paB[:], attg[CH:128, :], r1v[:, sl],
                                     start=True, stop=True)
                    nc.scalar.activation(x0v[:, sl], paA[:], ACT.Relu,
                                         bias=attbt[:, 0:1])
                    nc.scalar.activation(x1v[:, sl], paB[:], ACT.Relu,
                                         bias=attbt[:, 0:1])
            # D = recon - xcr (in place; this is what x_all subtracts)
            nc.vector.tensor_tensor(
                recon[:, c * XPC : (c + 1) * XPC, :],
                recon[:, c * XPC : (c + 1) * XPC, :], xcr[:], OP.subtract,
            )
            # gather-subtract batch c: x_all -= D[gathered]
            for hf, dst0, src_hf, src0, take in gather_plan[c]:
                pb = hf * CH
                pbi = src_hf * CH
                if pbi != pb:
                    assert take <= j0
                    xstage = small.tile([128, j0, w], BF16, tag="xstage",
                                        name="xstage")
                    nc.sync.dma_start(
                        xstage[pb : pb + CH, 0:take, :],
                        recon[pbi : pbi + CH, src0 : src0 + take, :],
                    )
                    srct, srow, spb = xstage, 0, pb
                else:
                    srct, srow, spb = recon, src0, pbi
                for co, cin, cl in col_runs:
                    src = srct[spb : spb + CH, srow : srow + take, cin : cin + cl]
                    dst = x_all[pb : pb + CH, 1 + dst0 : 1 + dst0 + take,
                                1 + co : 1 + co + cl]
                    eng = nc.vector if eng_flip[0] % 2 == 0 else nc.gpsimd
                    eng_flip[0] += 1
                    eng.tensor_tensor(dst, dst, src, OP.subtract)
            try_emit(c)

        # cross-half halo rows (x_all fully assembled now)
        nc.sync.dma_start(x_all[CH:128, 0, :], x_all[0:CH, hh, :])
        nc.sync.dma_start(x_all[0:CH, hh + 1, :], x_all[CH:128, 1, :])
        try_emit(99)
        assert not availq[0] and not availq[1] and not conv2_left

        lowp.__exit__(None, None, None)
        po1.release()
        prec.release()
        pxa.release()

    nc.finalize()
    return nc


_NC_CACHE = {}


def _get_nc(H=384, W=384, debug=False):
    key = (H, W, debug)
    if key not in _NC_CACHE:
        _NC_CACHE[key] = build_nc(H=H, W=W, debug=debug)
    return _NC_CACHE[key]


def kernel(x, w1, b1, w2, b2, att_conv_w, att_conv_b, fc1_w, fc2_w):
    x = np.ascontiguousarray(np.asarray(x, np.float32))
    B = x.shape[0]
    nc = _get_nc(x.shape[2], x.shape[3])
    shared = {
        "w1": np.ascontiguousarray(np.asarray(w1, np.float32)),
        "b1": np.ascontiguousarray(np.asarray(b1, np.float32)),
        "w2": np.ascontiguousarray(np.asarray(w2, np.float32)),
        "b2": np.ascontiguousarray(np.asarray(b2, np.float32)),
        "att_conv_w": np.ascontiguousarray(np.asarray(att_conv_w, np.float32)),
        "att_conv_b": np.ascontiguousarray(np.asarray(att_conv_b, np.float32)),
        "fc1_w": np.ascontiguousarray(np.asarray(fc1_w, np.float32)),
        "fc2_w": np.ascontiguousarray(np.asarray(fc2_w, np.float32)),
    }
    in_maps = [dict(shared, x=np.ascontiguousarray(x[i])) for i in range(B)]
    res = run_bass_kernel_spmd(nc, in_maps, core_ids=list(range(B)))
    return np.stack([res.results[i]["out"] for i in range(B)], axis=0)
